# revision 1
# baseline (speedup 1.0000x reference)
"""DeepseekV2 MLA attention (B=1, S=2048, H=4096, NH=32) on 8 TRN2 cores.

Sharding: tensor-parallel over heads (4 heads/core).  The q_a projection +
RMSNorm runs data-parallel over sequence (each core does its 256-row slice)
and is AllGathered; the (cheaper) kv_a front is replicated per core so the
K/V projections can proceed while the AllGather is in flight.  Each core
emits a partial output projection (its head slice of Wo); the host sums the
8 partials.

All matmul operands are pre-transposed/packed on the HOST into T-layout
([feature, seq]) so the PE always contracts over the partition dim with zero
on-device transposes.  RMSNorm ln weights and the softmax scale are folded
into Wqb/Wkvb host-side.  Attention runs as logits^T [k, q]: softmax over
the partition axis via ones-matmul denominators, no max subtraction (logits
are O(5) for randn inputs), mask applied as data (causal tiles skipped only
when the host verifies the mask is exactly causal).

Matmuls run in float32r (full-rate PE; ~3e-4 rel err end to end).
"""

import ctypes
import os
import numpy as np

import concourse.bass as bass
import concourse.mybir as mybir
from concourse.tile import TileContext
import concourse.bass_utils as bass_utils
from concourse.bass_utils import run_bass_kernel_spmd

bass_utils.upload_artifacts = lambda tmpdir: tmpdir  # no artifact bucket here

S = 2048
H = 4096
NCORES = 8
NHC = 4            # heads per core
NOPE, ROPE, VD = 128, 64, 128
QHD = NOPE + ROPE  # 192
QLR, KVLR = 1536, 512
BASE = 10000.0
EPS = 1e-6
SCALE = QHD ** -0.5
P = 128
SC = 512           # seq chunk (local phases)
SLC = S // NCORES  # 256, per-core front slice
NSC = S // SC      # 4
NKB = S // P       # 16 key blocks
FR = mybir.dt.float32r
F32 = mybir.dt.float32
AF = mybir.ActivationFunctionType

N_KI = H // P      # 32 front contraction tiles
NQB = QLR // P     # 12
NKVB = KVLR // P   # 4
FB_W = [P] * NQB + [P] * NKVB + [ROPE]  # 17 front output blocks
N_FB = len(FB_W)


def axon_reset():
    import jax
    jax.devices()
    lib = ctypes.CDLL('/opt/axon/libaxon_pjrt.so')
    lib.axon_reset.restype = ctypes.c_int64
    return lib.axon_reset()


def split_multiwaits(nc, cap=1):
    """This walrus pin allows only `cap` sync-waits per instruction; spill
    extras onto same-engine NoOps inserted just before the instruction."""
    for f in nc.m.functions:
        for b in f.blocks:
            li = b.instructions
            out = []
            changed = False
            for inst in list(li):
                si = getattr(inst, "sync_info", None)
                waits = list(si.on_wait) if si is not None and si.on_wait else []
                if len(waits) > cap:
                    changed = True
                    extra, keep = waits[:-cap], waits[-cap:]
                    for j in range(0, len(extra), cap):
                        out.append(mybir.InstNoOp(
                            name=nc.get_next_instruction_name(),
                            engine=inst.engine, ins=[], outs=[],
                            sync_info=mybir.SyncInfo(
                                on_wait=extra[j:j + cap], on_update=[]),
                            bass_nofuse=True,
                        ))
                    inst.sync_info = mybir.SyncInfo(
                        on_wait=keep, on_update=list(si.on_update))
                out.append(inst)
            if changed:
                li[:] = out


def build(causal: bool) -> bass.Bass:
    nc = bass.Bass()
    hT = nc.declare_dram_parameter("hT", [H, S], F32, isOutput=False)
    hTs = nc.declare_dram_parameter("hTs", [H, SLC], F32, isOutput=False)
    maskT = nc.declare_dram_parameter("maskT", [S, S], F32, isOutput=False)
    Wp = nc.declare_dram_parameter("Wp", [P, N_FB * N_KI * P], F32, isOutput=False)
    Wqb_p = nc.declare_dram_parameter("Wqb_p", [P, NQB * NHC * QHD], F32, isOutput=False)
    Wkvb_p = nc.declare_dram_parameter("Wkvb_p", [P, NKVB * NHC * (NOPE + VD)], F32, isOutput=False)
    Wo_p = nc.declare_dram_parameter("Wo_p", [P, NKVB * H], F32, isOutput=False)
    cq = nc.declare_dram_parameter("cq", [ROPE, S], F32, isOutput=False)
    sq = nc.declare_dram_parameter("sq", [ROPE, S], F32, isOutput=False)
    outT = nc.declare_dram_parameter("outT", [H, S], F32, isOutput=True)

    Wp3 = Wp.rearrange("p (fk w) -> p fk w", w=P)        # [P, 17*32, 128]
    Wqb3 = Wqb_p.rearrange("p (k w) -> p k w", k=NQB)    # [P, 12, 768]
    Wkvb3 = Wkvb_p.rearrange("p (k w) -> p k w", k=NKVB)  # [P, 4, 1024]
    Wo3 = Wo_p.rearrange("p (k w) -> p k w", k=NKVB)     # [P, 4, 4096]

    def fr(ap):
        return ap.bitcast(FR)

    with TileContext(nc) as tc:
        with (
            tc.tile_pool(name="dram", bufs=1, space="DRAM") as dpool,
            tc.tile_pool(name="const", bufs=1) as cpool,
        ):
            kvnT = dpool.tile([KVLR, S], F32)
            qnT = dpool.tile([NHC * NOPE, S], F32)
            qrT = dpool.tile([NHC * ROPE, S], F32)
            kpeT = dpool.tile([ROPE, S], F32)
            onT = dpool.tile([NHC * VD, S], F32)
            cc_q_in = dpool.tile([QLR, SLC], F32)
            cc_q_out = dpool.tile([NCORES, QLR, SLC], F32, addr_space="Shared")
            ones_f = cpool.tile([P, 1], F32)
            nc.vector.memset(ones_f[:], 1.0)
            ones_rf = cpool.tile([1, P], F32)
            nc.vector.memset(ones_rf[:], 1.0)
            ones_t = cpool.tile([P, 1], FR)
            nc.scalar.copy(ones_t[:], ones_f[:])
            ones_row = cpool.tile([1, P], FR)
            nc.scalar.copy(ones_row[:], ones_rf[:])

            # ------------- Phase 1: front projections + RMSNorm + k rope
            with (
                tc.tile_pool(name="hcol", bufs=1) as hpool,
                tc.tile_pool(name="wfr", bufs=2) as wpool,
                tc.tile_pool(name="raw", bufs=1) as rpool,
                tc.tile_pool(name="nrm", bufs=2) as npool,
                tc.tile_pool(name="ckr", bufs=1) as ckpool,
                tc.tile_pool(name="ps", bufs=3, space="PSUM") as pspool,
                tc.tile_pool(name="ps1", bufs=1, space="PSUM") as ps1pool,
            ):
                # --- 1q: q_a on the local 256-col slice, then AllGather
                hqs = []
                for ki in range(N_KI):
                    ht = hpool.tile([P, SLC], FR, tag=f"h{ki}", name=f"hq{ki}")
                    nc.gpsimd.dma_start(out=ht[:], in_=hTs[ki * P:(ki + 1) * P, :])
                    hqs.append(ht)
                qraws = []
                sq_qp = ps1pool.tile([1, SLC], F32, tag="sq_q")
                for fb in range(NQB):
                    wt = wpool.tile([P, N_KI, P], FR, tag="w", name=f"wq{fb}")
                    nc.gpsimd.dma_start(
                        out=wt[:], in_=Wp3[:, fb * N_KI:(fb + 1) * N_KI, :])
                    ps = pspool.tile([P, SLC], F32, tag="ps", name=f"psq{fb}")
                    for ki in range(N_KI):
                        nc.tensor.matmul(ps[:], lhsT=fr(wt[:, ki, :]), rhs=hqs[ki][:],
                                         start=(ki == 0), stop=(ki == N_KI - 1))
                    raw = rpool.tile([P, SLC], F32, tag=f"r{fb}", name=f"rq{fb}")
                    nc.scalar.copy(raw[:], ps[:])
                    qraws.append(raw)
                    sqt = npool.tile([P, SLC], FR, tag="sqt", name=f"sqtq{fb}")
                    nc.vector.tensor_mul(sqt[:], raw[:], raw[:])
                    nc.tensor.matmul(sq_qp[:], lhsT=ones_t[:], rhs=sqt[:],
                                     start=(fb == 0), stop=(fb == NQB - 1))
                ms = npool.tile([1, SLC], F32, tag="ms", name="msq")
                nc.scalar.activation(ms[:], sq_qp[:], AF.Copy, scale=1.0 / QLR, bias=EPS)
                rc = npool.tile([1, SLC], F32, tag="rc", name="rcq")
                nc.vector.reciprocal(rc[:], ms[:])
                rs = npool.tile([1, SLC], FR, tag="rs", name="rsq")
                nc.scalar.activation(rs[:], rc[:], AF.Sqrt)
                bps = ps1pool.tile([P, SLC], F32, tag="bps", name="bpsq")
                nc.tensor.matmul(bps[:], lhsT=ones_row[:], rhs=rs[:], start=True, stop=True)
                rb = npool.tile([P, SLC], F32, tag="rb", name="rbq")
                nc.scalar.copy(rb[:], bps[:])
                for j in range(NQB):
                    nt = npool.tile([P, SLC], F32, tag="nt", name=f"ntq{j}")
                    nc.vector.tensor_mul(nt[:], qraws[j][:], rb[:])
                    nc.gpsimd.dma_start(out=cc_q_in[j * P:(j + 1) * P, :], in_=nt[:])
                nc.gpsimd.collective_compute(
                    "AllGather", mybir.AluOpType.bypass,
                    replica_groups=[list(range(NCORES))],
                    ins=[cc_q_in.opt()], outs=[cc_q_out.opt()])

                # --- 1kv: kv_a + rope over the full sequence (replicated)
                ck_t = ckpool.tile([ROPE, S], F32, tag="ck")
                sk_t = ckpool.tile([ROPE, S], F32, tag="sk")
                nc.gpsimd.dma_start(out=ck_t[:], in_=cq[:, :])
                nc.gpsimd.dma_start(out=sk_t[:], in_=sq[:, :])
                for sc in range(NSC):
                    ssl = slice(sc * SC, (sc + 1) * SC)
                    hts = []
                    for ki in range(N_KI):
                        ht = hpool.tile([P, SC], FR, tag=f"h{ki}", name=f"hk{ki}_{sc}")
                        nc.gpsimd.dma_start(out=ht[:], in_=hT[ki * P:(ki + 1) * P, ssl])
                        hts.append(ht)
                    raws = []
                    sq_kv = ps1pool.tile([1, SC], F32, tag="sq_kv")
                    for fbi, fb in enumerate(range(NQB, N_FB)):
                        w = FB_W[fb]
                        wt = wpool.tile([P, N_KI, P], FR, tag="w", name=f"wk{fb}_{sc}")
                        nc.gpsimd.dma_start(
                            out=wt[:], in_=Wp3[:, fb * N_KI:(fb + 1) * N_KI, :])
                        ps = pspool.tile([P, SC], F32, tag="ps", name=f"psk{fb}_{sc}")
                        for ki in range(N_KI):
                            nc.tensor.matmul(ps[:w, :], lhsT=fr(wt[:, ki, :w]), rhs=hts[ki][:],
                                             start=(ki == 0), stop=(ki == N_KI - 1))
                        raw = rpool.tile([P, SC], F32, tag=f"r{fb}", name=f"rk{fb}_{sc}")
                        nc.scalar.copy(raw[:w, :], ps[:w, :])
                        raws.append(raw)
                        if fb < NQB + NKVB:
                            sqt = npool.tile([P, SC], FR, tag="sqt", name=f"sqtk{fb}_{sc}")
                            nc.vector.tensor_mul(sqt[:], raw[:], raw[:])
                            nc.tensor.matmul(sq_kv[:], lhsT=ones_t[:], rhs=sqt[:],
                                             start=(fb == NQB), stop=(fb == NQB + NKVB - 1))
                    ms = npool.tile([1, SC], F32, tag="ms", name=f"msk{sc}")
                    nc.scalar.activation(ms[:], sq_kv[:], AF.Copy, scale=1.0 / KVLR, bias=EPS)
                    rc = npool.tile([1, SC], F32, tag="rc", name=f"rck{sc}")
                    nc.vector.reciprocal(rc[:], ms[:])
                    rs = npool.tile([1, SC], FR, tag="rs", name=f"rsk{sc}")
                    nc.scalar.activation(rs[:], rc[:], AF.Sqrt)
                    bps = ps1pool.tile([P, SC], F32, tag="bps", name=f"bpsk{sc}")
                    nc.tensor.matmul(bps[:], lhsT=ones_row[:], rhs=rs[:], start=True, stop=True)
                    rb = npool.tile([P, SC], F32, tag="rb", name=f"rbk{sc}")
                    nc.scalar.copy(rb[:], bps[:])
                    for j in range(NKVB):
                        nt = npool.tile([P, SC], F32, tag="nt", name=f"ntk{j}_{sc}")
                        nc.vector.tensor_mul(nt[:], raws[j][:], rb[:])
                        nc.gpsimd.dma_start(out=kvnT[j * P:(j + 1) * P, ssl], in_=nt[:])
                    kraw = raws[NKVB]
                    ksw = npool.tile([ROPE, SC], F32, tag="ksw", name=f"ksw{sc}")
                    nc.gpsimd.dma_start(out=ksw[0:32, :], in_=kraw[32:64, :])
                    nc.gpsimd.dma_start(out=ksw[32:64, :], in_=kraw[0:32, :])
                    ka = npool.tile([ROPE, SC], F32, tag="ka", name=f"ka{sc}")
                    nc.vector.tensor_mul(ka[:], kraw[:ROPE, :], ck_t[:, ssl])
                    kb_ = npool.tile([ROPE, SC], F32, tag="kb", name=f"kb{sc}")
                    nc.vector.tensor_mul(kb_[:], ksw[:], sk_t[:, ssl])
                    ko = npool.tile([ROPE, SC], F32, tag="ko", name=f"ko{sc}")
                    nc.vector.tensor_add(ko[:], ka[:], kb_[:])
                    nc.gpsimd.dma_start(out=kpeT[:, ssl], in_=ko[:])

            if True:
                # ------------- Phase 2a-kv: K_nope / V projections (local data,
                # runs while the q AllGather is in flight)
                kv2pool = tc.tile_pool(name="kv2", bufs=1)
                kv2 = kv2pool.__enter__()
                KN = [kv2.tile([NOPE, S], FR, tag=f"kn{h}", name=f"kn{h}") for h in range(NHC)]
                V = [kv2.tile([P, NHC, VD], FR, tag=f"v{sb}", name=f"v{sb}") for sb in range(NKB)]
                kpe_sb = kv2.tile([ROPE, S], FR, tag="kpe")
                nc.gpsimd.dma_start(out=kpe_sb[:], in_=kpeT[:, :])
                with (
                    tc.tile_pool(name="whk", bufs=1) as whpool,
                    tc.tile_pool(name="acol2", bufs=1) as apool,
                    tc.tile_pool(name="ps2k", bufs=2, space="PSUM") as ps2pool,
                ):
                    wkvb_t = whpool.tile([P, NKVB, NHC * (NOPE + VD)], FR, tag="wkvb")
                    nc.gpsimd.dma_start(out=wkvb_t[:], in_=Wkvb3[:, :, :])
                    for sc in range(NSC):
                        ssl = slice(sc * SC, (sc + 1) * SC)
                        kvc = []
                        for j in range(NKVB):
                            t = apool.tile([P, SC], FR, tag=f"kv{j}", name=f"kvc{j}_{sc}")
                            nc.gpsimd.dma_start(out=t[:], in_=kvnT[j * P:(j + 1) * P, ssl])
                            kvc.append(t)
                        for h in range(NHC):
                            koff = h * (NOPE + VD)
                            ps = ps2pool.tile([P, SC], F32, tag="p2", name=f"p2k{h}_{sc}")
                            for j in range(NKVB):
                                nc.tensor.matmul(ps[:], lhsT=fr(wkvb_t[:, j, koff:koff + NOPE]),
                                                 rhs=kvc[j][:],
                                                 start=(j == 0), stop=(j == NKVB - 1))
                            nc.scalar.copy(KN[h][:, ssl], ps[:])
                            for sb in range(SC // P):
                                psv = ps2pool.tile([P, VD], F32, tag="pv", name=f"pv{h}_{sc}_{sb}")
                                for j in range(NKVB):
                                    nc.tensor.matmul(
                                        psv[:], lhsT=fr(kvc[j][:, sb * P:(sb + 1) * P]),
                                        rhs=fr(wkvb_t[:, j, koff + NOPE:koff + NOPE + VD]),
                                        start=(j == 0), stop=(j == NKVB - 1))
                                nc.scalar.copy(V[sc * (SC // P) + sb][:, h, :], psv[:])

                # ------------- Phase 2a-q: Q projections + rope (consumes the
                # AllGathered q_a_n, rank-chunked)
                with (
                    tc.tile_pool(name="whq", bufs=1) as whpool,
                    tc.tile_pool(name="acol", bufs=1) as apool,
                    tc.tile_pool(name="rope", bufs=2) as ropepool,
                    tc.tile_pool(name="ps2", bufs=2, space="PSUM") as ps2pool,
                ):
                    wqb_t = whpool.tile([P, NQB, NHC * QHD], FR, tag="wqb")
                    nc.gpsimd.dma_start(out=wqb_t[:], in_=Wqb3[:, :, :])
                    cq_t = whpool.tile([ROPE, S], F32, tag="cq")
                    sq_t = whpool.tile([ROPE, S], F32, tag="sq")
                    nc.gpsimd.dma_start(out=cq_t[:], in_=cq[:, :])
                    nc.gpsimd.dma_start(out=sq_t[:], in_=sq[:, :])
                    for r in range(NCORES):
                        csl = slice(r * SLC, (r + 1) * SLC)
                        qac = []
                        for j in range(NQB):
                            t = apool.tile([P, SLC], FR, tag=f"qa{j}", name=f"qac{j}_{r}")
                            nc.gpsimd.dma_start(out=t[:], in_=cc_q_out[r, j * P:(j + 1) * P, :])
                            qac.append(t)
                        for h in range(NHC):
                            qoff = h * QHD
                            ps = ps2pool.tile([P, SLC], F32, tag="p2", name=f"p2q{h}_{r}")
                            for j in range(NQB):
                                nc.tensor.matmul(ps[:], lhsT=fr(wqb_t[:, j, qoff:qoff + NOPE]),
                                                 rhs=qac[j][:],
                                                 start=(j == 0), stop=(j == NQB - 1))
                            qns = ropepool.tile([NOPE, SLC], F32, tag="qns", name=f"qns{h}_{r}")
                            nc.scalar.copy(qns[:], ps[:])
                            nc.gpsimd.dma_start(out=qnT[h * NOPE:(h + 1) * NOPE, csl], in_=qns[:])
                            ps64 = ps2pool.tile([ROPE, SLC], F32, tag="p64", name=f"p64q{h}_{r}")
                            for j in range(NQB):
                                nc.tensor.matmul(ps64[:], lhsT=fr(wqb_t[:, j, qoff + NOPE:qoff + QHD]),
                                                 rhs=qac[j][:],
                                                 start=(j == 0), stop=(j == NQB - 1))
                            qraw = ropepool.tile([ROPE, SLC], F32, tag="qraw", name=f"qraw{h}_{r}")
                            nc.scalar.copy(qraw[:], ps64[:])
                            qsw = ropepool.tile([ROPE, SLC], F32, tag="qsw", name=f"qsw{h}_{r}")
                            nc.gpsimd.dma_start(out=qsw[0:32, :], in_=qraw[32:64, :])
                            nc.gpsimd.dma_start(out=qsw[32:64, :], in_=qraw[0:32, :])
                            qa_ = ropepool.tile([ROPE, SLC], F32, tag="qa_", name=f"qa_{h}_{r}")
                            nc.vector.tensor_mul(qa_[:], qraw[:], cq_t[:, csl])
                            qb_ = ropepool.tile([ROPE, SLC], F32, tag="qb_", name=f"qb_{h}_{r}")
                            nc.vector.tensor_mul(qb_[:], qsw[:], sq_t[:, csl])
                            qrs = ropepool.tile([ROPE, SLC], F32, tag="qrs", name=f"qrs{h}_{r}")
                            nc.vector.tensor_add(qrs[:], qa_[:], qb_[:])
                            nc.gpsimd.dma_start(out=qrT[h * ROPE:(h + 1) * ROPE, csl], in_=qrs[:])

                # ------------- Phase 2b: attention
                with (
                    tc.tile_pool(name="att", bufs=2) as attpool,
                    tc.tile_pool(name="den", bufs=1) as denpool,
                    tc.tile_pool(name="ps_o", bufs=1, space="PSUM") as psopool,
                    tc.tile_pool(name="ps_l", bufs=2, space="PSUM") as pslpool,
                    tc.tile_pool(name="ps_d", bufs=1, space="PSUM") as psdpool,
                ):
                    for qc in range(NSC):
                        qsl = slice(qc * SC, (qc + 1) * SC)
                        kb_hi = (qc * 4 + 4) if causal else NKB
                        ops = [psopool.tile([VD, SC], F32, tag=f"o{h}", name=f"o{h}_{qc}") for h in range(NHC)]
                        dens = [denpool.tile([P, SC], FR, tag=f"d{h}", name=f"d{h}_{qc}") for h in range(NHC)]
                        qn_s, qr_s = [], []
                        for h in range(NHC):
                            qt = denpool.tile([NOPE, SC], FR, tag=f"qns{h}", name=f"qnl{h}_{qc}")
                            nc.gpsimd.dma_start(out=qt[:], in_=qnT[h * NOPE:(h + 1) * NOPE, qsl])
                            qn_s.append(qt)
                            qt2 = denpool.tile([ROPE, SC], FR, tag=f"qrs{h}", name=f"qrl{h}_{qc}")
                            nc.gpsimd.dma_start(out=qt2[:], in_=qrT[h * ROPE:(h + 1) * ROPE, qsl])
                            qr_s.append(qt2)
                        for kb in range(kb_hi):
                            ksl = slice(kb * P, (kb + 1) * P)
                            mt = attpool.tile([P, SC], F32, tag="mt", name=f"mt{qc}_{kb}")
                            nc.gpsimd.dma_start(out=mt[:], in_=maskT[ksl, qsl])
                            for h in range(NHC):
                                pl = pslpool.tile([P, SC], F32, tag="pl", name=f"pl{qc}_{kb}_{h}")
                                nc.tensor.matmul(pl[:], lhsT=KN[h][:, ksl], rhs=qn_s[h][:],
                                                 start=True, stop=False)
                                nc.tensor.matmul(pl[:], lhsT=kpe_sb[:, ksl], rhs=qr_s[h][:],
                                                 start=False, stop=True)
                                pe_ = attpool.tile([P, SC], F32, tag="pe", name=f"pe{qc}_{kb}_{h}")
                                nc.vector.tensor_add(pe_[:], pl[:], mt[:])
                                px = attpool.tile([P, SC], FR, tag="px", name=f"px{qc}_{kb}_{h}")
                                nc.scalar.activation(px[:], pe_[:], AF.Exp)
                                if kb == 0:
                                    nc.vector.tensor_copy(dens[h][:], px[:])
                                else:
                                    nc.vector.tensor_add(dens[h][:], dens[h][:], px[:])
                                nc.tensor.matmul(ops[h][:], lhsT=fr(V[kb][:, h, :]), rhs=px[:],
                                                 start=(kb == 0), stop=(kb == kb_hi - 1))
                        for h in range(NHC):
                            dps = psdpool.tile([1, SC], F32, tag="dps", name=f"dps{qc}_{h}")
                            nc.tensor.matmul(dps[:], lhsT=ones_t[:], rhs=dens[h][:],
                                             start=True, stop=True)
                            dsb = attpool.tile([1, SC], F32, tag="dsb", name=f"dsb{qc}_{h}")
                            nc.scalar.copy(dsb[:], dps[:])
                            rcp = attpool.tile([1, SC], FR, tag="rcp", name=f"rcp{qc}_{h}")
                            with nc.allow_low_precision(reason="f32r rounding for broadcast matmul"):
                                nc.vector.reciprocal(rcp[:], dsb[:])
                            bps2 = psdpool.tile([VD, SC], F32, tag="bps2", name=f"bps2{qc}_{h}")
                            nc.tensor.matmul(bps2[:], lhsT=ones_row[:], rhs=rcp[:],
                                             start=True, stop=True)
                            rbb = attpool.tile([VD, SC], F32, tag="rbb", name=f"rbb{qc}_{h}")
                            nc.scalar.copy(rbb[:], bps2[:])
                            on_ = attpool.tile([VD, SC], F32, tag="on", name=f"on{qc}_{h}")
                            nc.vector.tensor_mul(on_[:], ops[h][:], rbb[:])
                            nc.gpsimd.dma_start(out=onT[h * VD:(h + 1) * VD, qsl], in_=on_[:])
                kv2pool.__exit__(None, None, None)

            # ------------- Phase 3: output projection (partial over head slice)
            with (
                tc.tile_pool(name="wo", bufs=1) as wopool,
                tc.tile_pool(name="oc", bufs=1) as ocpool,
                tc.tile_pool(name="oo", bufs=3) as oopool,
                tc.tile_pool(name="po", bufs=3, space="PSUM") as popool,
            ):
                wo_t = wopool.tile([P, NKVB, H], FR, tag="wo")
                nc.gpsimd.dma_start(out=wo_t[:], in_=Wo3[:, :, :])
                for sc in range(NSC):
                    ssl = slice(sc * SC, (sc + 1) * SC)
                    ocs = []
                    for j in range(NKVB):
                        t = ocpool.tile([P, SC], FR, tag=f"oc{j}", name=f"oc{j}_{sc}")
                        nc.gpsimd.dma_start(out=t[:], in_=onT[j * P:(j + 1) * P, ssl])
                        ocs.append(t)
                    for ho in range(H // P):
                        ps = popool.tile([P, SC], F32, tag="po", name=f"po{sc}_{ho}")
                        for j in range(NKVB):
                            nc.tensor.matmul(ps[:], lhsT=fr(wo_t[:, j, ho * P:(ho + 1) * P]),
                                             rhs=ocs[j][:], start=(j == 0), stop=(j == NKVB - 1))
                        ot = oopool.tile([P, SC], F32, tag="ot", name=f"ot{sc}_{ho}")
                        nc.scalar.copy(ot[:], ps[:])
                        nc.gpsimd.dma_start(out=outT[ho * P:(ho + 1) * P, ssl], in_=ot[:])

    split_multiwaits(nc)
    return nc


def _pack_front(WqaT, WkvaT):
    """[4096, 1536+576] -> [128, 17*32, 128], zero-padded rope block."""
    Wfull = np.concatenate([WqaT, WkvaT], axis=1)
    out = np.zeros((P, N_FB * N_KI, P), np.float32)
    off = 0
    for fb, w in enumerate(FB_W):
        blk = Wfull[:, off:off + w].reshape(N_KI, P, w).transpose(1, 0, 2)
        out[:, fb * N_KI:(fb + 1) * N_KI, :w] = blk
        off += w
    return np.ascontiguousarray(out.reshape(P, -1))


def _pack_k(WT, nhw):
    """[K, nhw] -> [128, (K//128)*nhw]: k-tile-major packing of a T-layout weight."""
    K = WT.shape[0]
    t = WT.reshape(K // P, P, nhw).transpose(1, 0, 2).reshape(P, (K // P) * nhw)
    return np.ascontiguousarray(t, np.float32)


def _rope_tables():
    inv = 1.0 / (BASE ** (np.arange(0, ROPE, 2, dtype=np.float64) / ROPE))
    t = np.arange(S, dtype=np.float64)
    fr_ = np.outer(t, inv)
    emb = np.concatenate([fr_, fr_], axis=1)
    cos = np.cos(emb).T.astype(np.float32)
    sin = np.sin(emb).T.astype(np.float32)
    ssin = sin.copy()
    ssin[:32] *= -1.0
    return cos, ssin


def kernel(hidden_states, attention_mask, Wqa, qa_ln_w, Wqb, Wkva, kva_ln_w, Wkvb, Wo):
    hidden_states = np.asarray(hidden_states, np.float32)
    attention_mask = np.asarray(attention_mask, np.float32)
    Wqa = np.asarray(Wqa, np.float32)
    Wqb = np.asarray(Wqb, np.float32)
    Wkva = np.asarray(Wkva, np.float32)
    Wkvb = np.asarray(Wkvb, np.float32)
    Wo = np.asarray(Wo, np.float32)
    qa_ln_w = np.asarray(qa_ln_w, np.float32)
    kva_ln_w = np.asarray(kva_ln_w, np.float32)

    mask = attention_mask[0, 0]
    tril = np.tril(np.ones((S, S), bool))
    causal = bool(np.array_equal(mask, np.where(tril, 0.0, -1e9).astype(np.float32)))

    hT = np.ascontiguousarray(hidden_states[0].T)
    maskT = np.ascontiguousarray(mask.T)
    Wp = _pack_front(np.ascontiguousarray(Wqa.T), np.ascontiguousarray(Wkva.T))
    cos, ssin = _rope_tables()

    Wqb_eff = (Wqb * qa_ln_w[None, :]).astype(np.float32) * np.float32(SCALE)
    Wkvb_eff = (Wkvb * kva_ln_w[None, :]).astype(np.float32)

    in_maps = []
    for c in range(NCORES):
        hsl = slice(c * NHC * QHD, (c + 1) * NHC * QHD)
        ksl = slice(c * NHC * (NOPE + VD), (c + 1) * NHC * (NOPE + VD))
        osl = slice(c * NHC * VD, (c + 1) * NHC * VD)
        in_maps.append({
            "hT": hT, "maskT": maskT, "Wp": Wp,
            "hTs": np.ascontiguousarray(hT[:, c * SLC:(c + 1) * SLC]),
            "Wqb_p": _pack_k(np.ascontiguousarray(Wqb_eff[hsl].T), NHC * QHD),
            "Wkvb_p": _pack_k(np.ascontiguousarray(Wkvb_eff[ksl].T), NHC * (NOPE + VD)),
            "Wo_p": _pack_k(np.ascontiguousarray(Wo[:, osl].T), H),
            "cq": cos, "sq": ssin,
        })

    nc = build(causal)
    trace = bool(os.environ.get("KPROF"))
    res = run_bass_kernel_spmd(nc, in_maps, list(range(NCORES)), trace=trace)
    if trace:
        print(f"HW exec time: {res.exec_time_ns} ns (mean {res.mean_exec_time_ns}, "
              f"max core {res.max_exec_time_core_id})")
    acc = res.results[0]["outT"].copy()
    for c in range(1, NCORES):
        acc += res.results[c]["outT"]
    return np.ascontiguousarray(acc.T)[None, :, :].astype(np.float32)



# revision 19
# speedup vs baseline: 2.1790x; 2.1790x over previous
"""DeepseekV2 MLA attention (B=1, S=2048, H=4096, NH=32) on 8 TRN2 cores.

Sharding: tensor-parallel over heads (4 heads/core) for attention and the
up/out projections; data-parallel over sequence for the shared front
(q_a AND kv_a each run on the core's 256-token slice).  Two bf16
AllGathers distribute the compressed activations: ckv_n+roped-kpe
([576,2048], 2.4MB) and q_a_n ([1536,2048], 6.3MB).  Each core emits a
bf16 partial output projection (its head slice of Wo); the host sums the
8 partials in f32.

All matmuls run in bf16 (PSUM accumulate f32).  RMSNorm ln weights and
the softmax scale are folded into Wqb/Wkvb host-side.  Softmax runs over
the partition axis as logits^T [k, q]: denominators via ones-matmul, no
max subtraction (logits are O(1) for randn inputs).  Causal masking is
block-wise: off-diagonal key blocks skip the mask entirely; the 4
distinct diagonal 128x512 patterns are resident in SBUF.  The rope
contraction (64) is zero-padded to 128 partitions (K<128 matmuls are
~4x slower on HW).  K/V/Q/attention-out tiles all stay in SBUF.
"""

import ctypes
import os
import numpy as np

import concourse.bass as bass
import concourse.mybir as mybir
from concourse.tile import TileContext
import concourse.bass_utils as bass_utils
from concourse.bass_utils import run_bass_kernel_spmd

bass_utils.upload_artifacts = lambda tmpdir: tmpdir  # no artifact bucket here

S = 2048
H = 4096
NCORES = 8
NHC = 4            # heads per core
NPAIR = 2          # head pairs per core
NOPE, ROPE, VD = 128, 64, 128
QHD = NOPE + ROPE  # 192
QLR, KVLR = 1536, 512
BASE = 10000.0
EPS = 1e-6
SCALE = QHD ** -0.5
P = 128
SC = 512           # seq chunk for attention / K / Wo phases
SLC = S // NCORES  # 256, per-core front slice
NSC = S // SC      # 4
NKB = S // P       # 16 key blocks
BF = mybir.dt.bfloat16
FR = mybir.dt.float32r
F32 = mybir.dt.float32
AF = mybir.ActivationFunctionType

N_KI = H // P      # 32 front contraction tiles
NQB = QLR // P     # 12
NKVB = KVLR // P   # 4
# front output blocks: 4x kv(128), 1x rope(64 + 64 pad), 12x q(128)
N_FB = NKVB + 1 + NQB   # 17
FB_KV0, FB_ROPE, FB_Q0 = 0, NKVB, NKVB + 1


def axon_reset():
    import jax
    jax.devices()
    lib = ctypes.CDLL('/opt/axon/libaxon_pjrt.so')
    lib.axon_reset.restype = ctypes.c_int64
    return lib.axon_reset()


def split_multiwaits(nc, cap=1):
    """Allow only `cap` sync-waits per instruction; spill extras onto
    same-engine NoOps inserted just before the instruction."""
    for f in nc.m.functions:
        for b in f.blocks:
            li = b.instructions
            out = []
            changed = False
            for inst in list(li):
                si = getattr(inst, "sync_info", None)
                waits = list(si.on_wait) if si is not None and si.on_wait else []
                if len(waits) > cap:
                    changed = True
                    extra, keep = waits[:-cap], waits[-cap:]
                    for j in range(0, len(extra), cap):
                        out.append(mybir.InstNoOp(
                            name=nc.get_next_instruction_name(),
                            engine=inst.engine, ins=[], outs=[],
                            sync_info=mybir.SyncInfo(
                                on_wait=extra[j:j + cap], on_update=[]),
                            bass_nofuse=True,
                        ))
                    inst.sync_info = mybir.SyncInfo(
                        on_wait=keep, on_update=list(si.on_update))
                out.append(inst)
            if changed:
                li[:] = out


def build(causal: bool) -> bass.Bass:
    nc = bass.Bass()
    hs = nc.declare_dram_parameter("hs", [H, SLC], BF, isOutput=False)
    Wf = nc.declare_dram_parameter("Wf", [P, N_FB * N_KI * P], BF, isOutput=False)
    Wqb_p = nc.declare_dram_parameter("Wqb_p", [P, 3 * NPAIR * NQB * P], BF, isOutput=False)
    Wk_p = nc.declare_dram_parameter("Wk_p", [P, NKVB * NHC * P], BF, isOutput=False)
    Wv_p = nc.declare_dram_parameter("Wv_p", [P, NKVB * NHC * VD], BF, isOutput=False)
    Wo_p = nc.declare_dram_parameter("Wo_p", [P, (H // P) * NKVB * P], BF, isOutput=False)
    csF = nc.declare_dram_parameter("csF", [P, S], F32, isOutput=False)
    ssF = nc.declare_dram_parameter("ssF", [P, S], F32, isOutput=False)
    cs_loc = nc.declare_dram_parameter("cs_loc", [ROPE, SLC], F32, isOutput=False)
    ss_loc = nc.declare_dram_parameter("ss_loc", [ROPE, SLC], F32, isOutput=False)
    if causal:
        maskd = nc.declare_dram_parameter("maskd", [P, 4 * SC], F32, isOutput=False)
    else:
        maskT = nc.declare_dram_parameter("maskT", [S, S], F32, isOutput=False)
    out_p = nc.declare_dram_parameter("out_p", [H, S], BF, isOutput=True)

    Wf4 = Wf.rearrange("p (fb ki w) -> p fb ki w", fb=N_FB, ki=N_KI)
    Wqb4 = Wqb_p.rearrange("p (ob j w) -> p ob j w", ob=3 * NPAIR, j=NQB)
    Wk3 = Wk_p.rearrange("p (j w) -> p j w", j=NKVB)
    Wv3 = Wv_p.rearrange("p (j w) -> p j w", j=NKVB)
    Wo4 = Wo_p.rearrange("p (ho j w) -> p ho j w", ho=H // P, j=NKVB)

    with TileContext(nc) as tc:
        with (
            tc.tile_pool(name="dram", bufs=1, space="DRAM") as dpool,
            tc.tile_pool(name="const", bufs=1) as cpool,
        ):
            cc1_in = dpool.tile([KVLR + ROPE, SLC], BF)
            cc1_out = dpool.tile([NCORES, KVLR + ROPE, SLC], BF, addr_space="Shared")
            cc2_in = dpool.tile([QLR, SLC], BF)
            cc2_out = dpool.tile([NCORES, QLR, SLC], BF, addr_space="Shared")

            # constants
            ones_f = cpool.tile([P, 1], F32)
            nc.vector.memset(ones_f[:], 1.0)
            ones_rf = cpool.tile([1, P], F32)
            nc.vector.memset(ones_rf[:], 1.0)
            onesc_fr = cpool.tile([P, 1], FR)
            nc.scalar.copy(onesc_fr[:], ones_f[:])
            ones_row_fr = cpool.tile([1, P], FR)
            nc.scalar.copy(ones_row_fr[:], ones_rf[:])
            ones_bf = cpool.tile([P, 1], BF)
            nc.scalar.copy(ones_bf[:], ones_f[:])

            # rope tables + mask, loaded once
            cs_t = cpool.tile([P, S], F32)
            ss_t = cpool.tile([P, S], F32)
            nc.sync.dma_start(out=cs_t[:], in_=csF[:, :])
            nc.sync.dma_start(out=ss_t[:], in_=ssF[:, :])
            csl_t = cpool.tile([ROPE, SLC], F32)
            ssl_t = cpool.tile([ROPE, SLC], F32)
            nc.sync.dma_start(out=csl_t[:], in_=cs_loc[:, :])
            nc.sync.dma_start(out=ssl_t[:], in_=ss_loc[:, :])
            if causal:
                maskd_t = cpool.tile([P, 4, SC], F32)
                nc.sync.dma_start(out=maskd_t[:], in_=maskd.rearrange(
                    "p (d w) -> p d w", d=4)[:, :, :])

            # persistent activations (bf16, SBUF-resident)
            KN = [cpool.tile([NOPE, S], BF, tag=f"kn{h}", name=f"kn{h}") for h in range(NHC)]
            # kpe with zero-padded 128 contraction: lo = rows 0:64 (even
            # heads), hi = rows 64:128 (odd heads); pair-rope rhs QRP keeps
            # each head's rope on its natural partition half.
            kpe_lo = cpool.tile([P, S], BF, tag="kpelo")
            kpe_hi = cpool.tile([P, S], BF, tag="kpehi")
            nc.gpsimd.memset(kpe_lo[:], 0.0)
            nc.gpsimd.memset(kpe_hi[:], 0.0)
            V = [cpool.tile([P, NHC * VD], BF, tag=f"v{kb}", name=f"v{kb}") for kb in range(NKB)]
            QN = [cpool.tile([NOPE, S], BF, tag=f"qn{h}", name=f"qn{h}") for h in range(NHC)]
            QRP = [cpool.tile([P, S], BF, tag=f"qrp{pr}", name=f"qrp{pr}") for pr in range(NPAIR)]
            ON = [cpool.tile([VD, S], BF, tag=f"on{h}", name=f"on{h}") for h in range(NHC)]

            # ---------------- Phase F: front projections (local 256 cols)
            with (
                tc.tile_pool(name="hcol", bufs=1) as hpool,
                tc.tile_pool(name="wfr", bufs=2) as wfpool,
                tc.tile_pool(name="raw", bufs=1) as rpool,
                tc.tile_pool(name="nrm", bufs=2) as npool,
                tc.tile_pool(name="psf", bufs=3, space="PSUM") as pspool,
                tc.tile_pool(name="psf1", bufs=1, space="PSUM") as ps1pool,
            ):
                hts = []
                for ki in range(N_KI):
                    ht = hpool.tile([P, SLC], BF, tag=f"h{ki}", name=f"h{ki}")
                    nc.gpsimd.dma_start(out=ht[:], in_=hs[ki * P:(ki + 1) * P, :])
                    hts.append(ht)

                def front_block(fb, w, raws, sq_ps, sq_first, sq_last):
                    wt = wfpool.tile([P, N_KI, P], BF, tag="wf", name=f"wf{fb}")
                    nc.gpsimd.dma_start(out=wt[:], in_=Wf4[:, fb, :, :])
                    ps = pspool.tile([P, SLC], F32, tag="ps", name=f"psf{fb}")
                    for ki in range(N_KI):
                        nc.tensor.matmul(ps[:w, :], lhsT=wt[:, ki, :w], rhs=hts[ki][:],
                                         start=(ki == 0), stop=(ki == N_KI - 1))
                    raw = rpool.tile([P, SLC], F32, tag=f"r{fb}", name=f"raw{fb}")
                    nc.scalar.copy(raw[:w, :], ps[:w, :])
                    raws.append(raw)
                    if sq_ps is not None:
                        sqt = npool.tile([P, SLC], FR, tag="sqt", name=f"sqt{fb}")
                        nc.vector.tensor_mul(sqt[:], raw[:], raw[:])
                        nc.tensor.matmul(sq_ps[:], lhsT=onesc_fr[:], rhs=sqt[:],
                                         start=sq_first, stop=sq_last)

                def rmsnorm_bcast(sq_ps, dim, nm):
                    ms = npool.tile([1, SLC], F32, tag="ms", name=f"ms{nm}")
                    nc.scalar.activation(ms[:], sq_ps[:], AF.Copy,
                                         scale=1.0 / dim, bias=EPS)
                    rc = npool.tile([1, SLC], F32, tag="rc", name=f"rc{nm}")
                    nc.vector.reciprocal(rc[:], ms[:])
                    rs = npool.tile([1, SLC], FR, tag="rs", name=f"rs{nm}")
                    nc.scalar.activation(rs[:], rc[:], AF.Sqrt)
                    bps = ps1pool.tile([P, SLC], F32, tag="bps", name=f"bps{nm}")
                    nc.tensor.matmul(bps[:], lhsT=ones_row_fr[:], rhs=rs[:],
                                     start=True, stop=True)
                    rb = npool.tile([P, SLC], F32, tag=f"rb{nm}", name=f"rb{nm}")
                    nc.vector.tensor_copy(rb[:], bps[:])
                    return rb

                # --- kv blocks + rope block first (feeds cc1 early)
                kv_raws = []
                sq_kv = ps1pool.tile([1, SLC], F32, tag="sqkv")
                for j in range(NKVB):
                    front_block(FB_KV0 + j, P, kv_raws, sq_kv, j == 0, j == NKVB - 1)
                front_block(FB_ROPE, ROPE, kv_raws, None, False, False)
                rb_kv = rmsnorm_bcast(sq_kv, KVLR, "kv")
                for j in range(NKVB):
                    nt = npool.tile([P, SLC], BF, tag="nt", name=f"ntkv{j}")
                    nc.vector.tensor_mul(nt[:], kv_raws[j][:], rb_kv[:])
                    nc.scalar.dma_start(out=cc1_in[j * P:(j + 1) * P, :], in_=nt[:])
                # kpe rope (local positions)
                kraw = kv_raws[NKVB]
                ksw = npool.tile([ROPE, SLC], F32, tag="ksw", name="ksw")
                nc.scalar.dma_start(out=ksw[0:32, :], in_=kraw[32:64, :])
                nc.scalar.dma_start(out=ksw[32:64, :], in_=kraw[0:32, :])
                ka = npool.tile([ROPE, SLC], F32, tag="ka", name="ka")
                nc.vector.tensor_mul(ka[:], kraw[:ROPE, :], csl_t[:])
                kb_ = npool.tile([ROPE, SLC], F32, tag="kb", name="kb")
                nc.vector.tensor_mul(kb_[:], ksw[:], ssl_t[:])
                ko = npool.tile([ROPE, SLC], BF, tag="ko", name="ko")
                nc.vector.tensor_add(ko[:], ka[:], kb_[:])
                nc.scalar.dma_start(out=cc1_in[KVLR:KVLR + ROPE, :], in_=ko[:])
                nc.gpsimd.collective_compute(
                    "AllGather", mybir.AluOpType.bypass,
                    replica_groups=[list(range(NCORES))],
                    ins=[cc1_in.opt()], outs=[cc1_out.opt()])

                # --- q blocks
                q_raws = []
                sq_q = ps1pool.tile([1, SLC], F32, tag="sqq")
                for j in range(NQB):
                    front_block(FB_Q0 + j, P, q_raws, sq_q, j == 0, j == NQB - 1)
                rb_q = rmsnorm_bcast(sq_q, QLR, "q")
                for j in range(NQB):
                    nt = npool.tile([P, SLC], BF, tag="nt", name=f"ntq{j}")
                    nc.vector.tensor_mul(nt[:], q_raws[j][:], rb_q[:])
                    nc.scalar.dma_start(out=cc2_in[j * P:(j + 1) * P, :], in_=nt[:])
                nc.gpsimd.collective_compute(
                    "AllGather", mybir.AluOpType.bypass,
                    replica_groups=[list(range(NCORES))],
                    ins=[cc2_in.opt()], outs=[cc2_out.opt()])

            # ---------------- Phase KV: K_nope / V projections (after cc1)
            with (
                tc.tile_pool(name="wkv", bufs=1) as wkvpool,
                tc.tile_pool(name="kvc", bufs=2) as kvcpool,
                tc.tile_pool(name="pskv", bufs=2, space="PSUM") as pskvpool,
            ):
                wk_t = wkvpool.tile([P, NKVB, NHC * P], BF, tag="wk")
                nc.gpsimd.dma_start(out=wk_t[:], in_=Wk3[:, :, :])
                wv_t = wkvpool.tile([P, NKVB, NHC * VD], BF, tag="wv")
                nc.gpsimd.dma_start(out=wv_t[:], in_=Wv3[:, :, :])
                for r in range(NCORES):
                    nc.gpsimd.dma_start(
                        out=kpe_lo[0:ROPE, r * SLC:(r + 1) * SLC],
                        in_=cc1_out[r, KVLR:KVLR + ROPE, :])
                    nc.gpsimd.dma_start(
                        out=kpe_hi[ROPE:P, r * SLC:(r + 1) * SLC],
                        in_=cc1_out[r, KVLR:KVLR + ROPE, :])
                for qc in range(NSC):
                    qsl = slice(qc * SC, (qc + 1) * SC)
                    kvc = []
                    for j in range(NKVB):
                        t = kvcpool.tile([P, SC], BF, tag=f"kv{j}", name=f"kvc{j}_{qc}")
                        for rr in range(2):
                            r = 2 * qc + rr
                            nc.gpsimd.dma_start(
                                out=t[:, rr * SLC:(rr + 1) * SLC],
                                in_=cc1_out[r, j * P:(j + 1) * P, :])
                        kvc.append(t)
                    for h in range(NHC):
                        ps = pskvpool.tile([P, SC], F32, tag="pk", name=f"pk{h}_{qc}")
                        for j in range(NKVB):
                            nc.tensor.matmul(ps[:], lhsT=wk_t[:, j, h * P:(h + 1) * P],
                                             rhs=kvc[j][:],
                                             start=(j == 0), stop=(j == NKVB - 1))
                        nc.scalar.copy(KN[h][:, qsl], ps[:])
                    for sbl in range(SC // P):
                        kb = qc * (SC // P) + sbl
                        psv = pskvpool.tile([P, NHC * VD], F32, tag="pv", name=f"pv{kb}")
                        for j in range(NKVB):
                            nc.tensor.matmul(
                                psv[:], lhsT=kvc[j][:, sbl * P:(sbl + 1) * P],
                                rhs=wv_t[:, j, :],
                                start=(j == 0), stop=(j == NKVB - 1))
                        nc.scalar.copy(V[kb][:], psv[:])

            # ---------------- Phase Q: Wqb up-projection + rope (after cc2)
            with (
                tc.tile_pool(name="wqb", bufs=1) as wqbpool,
                tc.tile_pool(name="qat", bufs=2) as qatpool,
                tc.tile_pool(name="rope", bufs=2) as ropepool,
                tc.tile_pool(name="psq", bufs=3, space="PSUM") as psqpool,
            ):
                wqb_t = wqbpool.tile([P, 3 * NPAIR, NQB, P], BF, tag="wqb")
                nc.gpsimd.dma_start(out=wqb_t[:], in_=Wqb4[:, :, :, :])
                for qc in range(NSC):
                    qsl = slice(qc * SC, (qc + 1) * SC)
                    qa = []
                    for j in range(NQB):
                        t = qatpool.tile([P, SC], BF, tag=f"qa{j}", name=f"qa{j}_{qc}")
                        for rr in range(2):
                            r = 2 * qc + rr
                            nc.sync.dma_start(
                                out=t[:, rr * SLC:(rr + 1) * SLC],
                                in_=cc2_out[r, j * P:(j + 1) * P, :])
                        qa.append(t)

                    def qmm(ob, nm):
                        ps = psqpool.tile([P, SC], F32, tag="pq", name=f"pq{nm}_{qc}")
                        for j in range(NQB):
                            nc.tensor.matmul(ps[:], lhsT=wqb_t[:, ob, j, :],
                                             rhs=qa[j][:],
                                             start=(j == 0), stop=(j == NQB - 1))
                        return ps

                    for pr in range(NPAIR):
                        h0, h1 = 2 * pr, 2 * pr + 1
                        ps = qmm(3 * pr + 0, f"n{h0}")
                        nc.scalar.copy(QN[h0][:, qsl], ps[:])
                        ps = qmm(3 * pr + 1, f"r{pr}")
                        qraw = ropepool.tile([P, SC], F32, tag="qraw", name=f"qraw{pr}_{qc}")
                        nc.vector.tensor_copy(qraw[:], ps[:])
                        qsw = ropepool.tile([P, SC], F32, tag="qsw", name=f"qsw{pr}_{qc}")
                        nc.sync.dma_start(out=qsw[0:32, :], in_=qraw[32:64, :])
                        nc.sync.dma_start(out=qsw[32:64, :], in_=qraw[0:32, :])
                        nc.sync.dma_start(out=qsw[64:96, :], in_=qraw[96:128, :])
                        nc.sync.dma_start(out=qsw[96:128, :], in_=qraw[64:96, :])
                        qa_ = ropepool.tile([P, SC], F32, tag="qa_", name=f"qa_{pr}_{qc}")
                        nc.vector.tensor_mul(qa_[:], qraw[:], cs_t[:, qsl])
                        qb_ = ropepool.tile([P, SC], F32, tag="qb_", name=f"qb_{pr}_{qc}")
                        nc.vector.tensor_mul(qb_[:], qsw[:], ss_t[:, qsl])
                        nc.vector.tensor_add(QRP[pr][:, qsl], qa_[:], qb_[:])
                        ps = qmm(3 * pr + 2, f"n{h1}")
                        nc.scalar.copy(QN[h1][:, qsl], ps[:])

            # ---------------- Phase A: attention
            with (
                tc.tile_pool(name="att", bufs=2) as attpool,
                tc.tile_pool(name="psl", bufs=2, space="PSUM") as pslpool,
                tc.tile_pool(name="pso", bufs=2, space="PSUM") as psopool,
                tc.tile_pool(name="psd", bufs=2, space="PSUM") as psdpool,
                tc.tile_pool(name="psb", bufs=1, space="PSUM") as psbpool,
            ):
                for qc in range(NSC):
                    qsl = slice(qc * SC, (qc + 1) * SC)
                    kb_hi = (qc + 1) * (SC // P) if causal else NKB
                    for h in range(NHC):
                        ops = psopool.tile([VD, SC], F32, tag="ops", name=f"o{qc}_{h}")
                        dps = psdpool.tile([1, SC], F32, tag="dps", name=f"d{qc}_{h}")
                        deferred = None
                        for kb in range(kb_hi):
                            ksl = slice(kb * P, (kb + 1) * P)
                            pl = pslpool.tile([P, SC], F32, tag="pl",
                                              name=f"pl{qc}_{h}_{kb}")
                            kpe_t = kpe_lo if h % 2 == 0 else kpe_hi
                            nc.tensor.matmul(pl[:], lhsT=KN[h][:, ksl],
                                             rhs=QN[h][:, qsl], start=True, stop=False)
                            nc.tensor.matmul(pl[:], lhsT=kpe_t[:, ksl],
                                             rhs=QRP[h // 2][:, qsl], start=False, stop=True)
                            if deferred is not None:
                                pxp, first = deferred
                                nc.tensor.matmul(dps[:], lhsT=ones_bf[:], rhs=pxp[:],
                                                 start=first, stop=False)
                                nc.tensor.matmul(ops[:], lhsT=V[kb - 1][:, h * VD:(h + 1) * VD],
                                                 rhs=pxp[:], start=first, stop=False)
                            px = attpool.tile([P, SC], BF, tag="px",
                                              name=f"px{qc}_{h}_{kb}")
                            if causal and kb >= qc * (SC // P):
                                d = kb - qc * (SC // P)
                                pe_ = attpool.tile([P, SC], F32, tag="pe",
                                                   name=f"pe{qc}_{h}_{kb}")
                                nc.vector.tensor_add(pe_[:], pl[:], maskd_t[:, d, :])
                                nc.scalar.activation(px[:], pe_[:], AF.Exp)
                            elif not causal:
                                mt = attpool.tile([P, SC], F32, tag="mt",
                                                  name=f"mt{qc}_{h}_{kb}")
                                nc.gpsimd.dma_start(out=mt[:], in_=maskT[ksl, qsl])
                                pe_ = attpool.tile([P, SC], F32, tag="pe",
                                                   name=f"pe{qc}_{h}_{kb}")
                                nc.vector.tensor_add(pe_[:], pl[:], mt[:])
                                nc.scalar.activation(px[:], pe_[:], AF.Exp)
                            else:
                                nc.scalar.activation(px[:], pl[:], AF.Exp)
                            deferred = (px, kb == 0)
                        pxp, first = deferred
                        nc.tensor.matmul(dps[:], lhsT=ones_bf[:], rhs=pxp[:],
                                         start=first, stop=True)
                        nc.tensor.matmul(ops[:], lhsT=V[kb_hi - 1][:, h * VD:(h + 1) * VD],
                                         rhs=pxp[:], start=first, stop=True)
                        dsb = attpool.tile([1, SC], F32, tag="dsb", name=f"ds{qc}_{h}")
                        nc.vector.tensor_copy(dsb[:], dps[:])
                        rcp = attpool.tile([1, SC], FR, tag="rcp", name=f"rc{qc}_{h}")
                        with nc.allow_low_precision(reason="f32r denominators"):
                            nc.vector.reciprocal(rcp[:], dsb[:])
                        bps2 = psbpool.tile([VD, SC], F32, tag="bps2", name=f"b{qc}_{h}")
                        nc.tensor.matmul(bps2[:], lhsT=ones_row_fr[:],
                                         rhs=rcp[:], start=True, stop=True)
                        rbb = attpool.tile([VD, SC], F32, tag="rbb", name=f"rb{qc}_{h}")
                        nc.vector.tensor_copy(rbb[:], bps2[:])
                        nc.vector.tensor_mul(ON[h][:, qsl], ops[:], rbb[:])

            # ---------------- Phase O: output projection (partial over head slice)
            with (
                tc.tile_pool(name="wo", bufs=2) as wopool,
                tc.tile_pool(name="oo", bufs=4) as oopool,
                tc.tile_pool(name="po", bufs=3, space="PSUM") as popool,
            ):
                for ho in range(H // P):
                    wo_t = wopool.tile([P, NKVB, P], BF, tag="wo", name=f"wo{ho}")
                    nc.sync.dma_start(out=wo_t[:], in_=Wo4[:, ho, :, :])
                    for sc in range(NSC):
                        ssl = slice(sc * SC, (sc + 1) * SC)
                        ps = popool.tile([P, SC], F32, tag="po", name=f"po{ho}_{sc}")
                        for j in range(NKVB):
                            nc.tensor.matmul(ps[:], lhsT=wo_t[:, j, :],
                                             rhs=ON[j][:, ssl],
                                             start=(j == 0), stop=(j == NKVB - 1))
                        ot = oopool.tile([P, SC], BF, tag="ot", name=f"ot{ho}_{sc}")
                        if (ho + sc) % 2 == 0:
                            nc.scalar.copy(ot[:], ps[:])
                        else:
                            nc.vector.tensor_copy(ot[:], ps[:])
                        nc.sync.dma_start(out=out_p[ho * P:(ho + 1) * P, ssl], in_=ot[:])

    split_multiwaits(nc)
    return nc


def _rope_tables():
    inv = 1.0 / (BASE ** (np.arange(0, ROPE, 2, dtype=np.float64) / ROPE))
    t = np.arange(S, dtype=np.float64)
    fr_ = np.outer(t, inv)
    emb = np.concatenate([fr_, fr_], axis=1)
    cos = np.cos(emb).T.astype(np.float32)          # [64, S]
    sin = np.sin(emb).T.astype(np.float32)
    ssin = sin.copy()
    ssin[:32] *= -1.0
    return cos, ssin


def _to_bf(a):
    return a.astype(mybir.dt.np(BF))


def prepare(hidden_states, attention_mask, Wqa, qa_ln_w, Wqb, Wkva, kva_ln_w, Wkvb, Wo):
    hidden_states = np.asarray(hidden_states, np.float32)
    attention_mask = np.asarray(attention_mask, np.float32)
    Wqa = np.asarray(Wqa, np.float32)
    Wqb = np.asarray(Wqb, np.float32)
    Wkva = np.asarray(Wkva, np.float32)
    Wkvb = np.asarray(Wkvb, np.float32)
    Wo = np.asarray(Wo, np.float32)
    qa_ln_w = np.asarray(qa_ln_w, np.float32)
    kva_ln_w = np.asarray(kva_ln_w, np.float32)

    mask = attention_mask[0, 0]
    tril = np.tril(np.ones((S, S), bool))
    causal = bool(np.array_equal(mask, np.where(tril, 0.0, -1e9).astype(np.float32)))

    hT = np.ascontiguousarray(hidden_states[0].T)          # [H, S]
    cos, ssin = _rope_tables()
    csF = np.ascontiguousarray(np.concatenate([cos, cos], axis=0))   # [128, S]
    ssF = np.ascontiguousarray(np.concatenate([ssin, ssin], axis=0))

    # front weight: [H, 2176] cols = kv(512) | rope(64)+pad(64) | q(1536)
    WT_all = np.concatenate([
        Wkva[:KVLR].T, Wkva[KVLR:].T, np.zeros((H, P - ROPE), np.float32),
        Wqa.T], axis=1)                                   # [4096, 2176]
    Wf = np.zeros((P, N_FB, N_KI, P), np.float32)
    for fb in range(N_FB):
        blk = WT_all[:, fb * P:(fb + 1) * P].reshape(N_KI, P, P)
        Wf[:, fb, :, :] = blk.transpose(1, 0, 2)
    Wf_b = _to_bf(Wf.reshape(P, -1))

    Wqb_eff = (Wqb * qa_ln_w[None, :]).astype(np.float32) * np.float32(SCALE)
    Wkvb_eff = (Wkvb * kva_ln_w[None, :]).astype(np.float32)

    def pack_lhsT(rows, ncols_blocks_shape):
        """rows: [Dout, K] weight slice -> lhsT pack [P, K//P, Dout] then
        reshape to ncols_blocks_shape with Dout blocked last."""
        WT = rows.T                                        # [K, Dout]
        K = WT.shape[0]
        t = WT.reshape(K // P, P, WT.shape[1]).transpose(1, 0, 2)  # [P, K//P, Dout]
        return t.reshape(ncols_blocks_shape)

    in_maps = []
    shared = {"Wf": Wf_b, "csF": csF, "ssF": ssF}
    if causal:
        d_idx = np.arange(P)[:, None] + np.zeros((1, SC), np.int64)
        q_idx = np.zeros((P, 1), np.int64) + np.arange(SC)[None, :]
        maskd = np.zeros((P, 4, SC), np.float32)
        for d in range(4):
            maskd[:, d, :] = np.where(d * P + d_idx <= q_idx, 0.0, -1e9)
        shared["maskd"] = np.ascontiguousarray(maskd.reshape(P, 4 * SC))
    else:
        shared["maskT"] = np.ascontiguousarray(mask.T)

    hT_b = _to_bf(hT)
    for c in range(NCORES):
        heads = range(c * NHC, (c + 1) * NHC)
        # Wqb pair-packed: per pair [nope_h0 | rope_h0;rope_h1 | nope_h1]
        rows = []
        for pr in range(NPAIR):
            h0 = c * NHC + 2 * pr
            h1 = h0 + 1
            rows.append(Wqb_eff[h0 * QHD:h0 * QHD + NOPE])
            rows.append(Wqb_eff[h0 * QHD + NOPE:h0 * QHD + QHD])
            rows.append(Wqb_eff[h1 * QHD + NOPE:h1 * QHD + QHD])
            rows.append(Wqb_eff[h1 * QHD:h1 * QHD + NOPE])
        Wqb_rows = np.concatenate(rows, axis=0)            # [768, 1536]
        # pack_lhsT gives [P, j, Dout]; we need [P, ob, j, w] ordering
        t = pack_lhsT(Wqb_rows, (P, NQB, 3 * NPAIR, P)).transpose(0, 2, 1, 3)
        Wqb_pk = np.ascontiguousarray(t.reshape(P, -1))

        Wk_rows = np.concatenate(
            [Wkvb_eff[h * (NOPE + VD):h * (NOPE + VD) + NOPE] for h in heads], axis=0)
        Wk_pk = pack_lhsT(Wk_rows, (P, NKVB, NHC * P))
        # lhsT layout wants [P, j, h*128+c] == t[P, j, Dout] directly
        Wk_pk = np.ascontiguousarray(Wk_pk.reshape(P, -1))

        Wv_rows = np.concatenate(
            [Wkvb_eff[h * (NOPE + VD) + NOPE:(h + 1) * (NOPE + VD)] for h in heads],
            axis=0)                                        # [512, 512]
        # rhs pack: [P(kvlr chunk), j, h*VD+c] = Wv_rows.T chunks
        Wv_pk = np.ascontiguousarray(pack_lhsT(Wv_rows, (P, NKVB, NHC * VD)).reshape(P, -1))

        Wo_cols = Wo[:, c * NHC * VD:(c + 1) * NHC * VD]   # [H, 512]
        t = pack_lhsT(Wo_cols, (P, NKVB, H))               # [P, j, H]
        t = t.reshape(P, NKVB, H // P, P).transpose(0, 2, 1, 3)  # [P, ho, j, w]
        Wo_pk = np.ascontiguousarray(t.reshape(P, -1))

        m = {
            "hs": np.ascontiguousarray(hT_b[:, c * SLC:(c + 1) * SLC]),
            "Wqb_p": _to_bf(Wqb_pk),
            "Wk_p": _to_bf(Wk_pk),
            "Wv_p": _to_bf(Wv_pk),
            "Wo_p": _to_bf(Wo_pk),
            "cs_loc": np.ascontiguousarray(cos[:, c * SLC:(c + 1) * SLC]),
            "ss_loc": np.ascontiguousarray(ssin[:, c * SLC:(c + 1) * SLC]),
        }
        m.update(shared)
        in_maps.append(m)
    return in_maps, causal


def kernel(**inputs):
    in_maps, causal = prepare(**inputs)
    nc = build(causal)
    trace = bool(os.environ.get("KPROF"))
    res = run_bass_kernel_spmd(nc, in_maps, list(range(NCORES)), trace=trace)
    if trace:
        print(f"HW exec time: {res.exec_time_ns} ns (mean {res.mean_exec_time_ns}, "
              f"max core {res.max_exec_time_core_id})")
    acc = np.zeros((H, S), np.float64)
    for c in range(NCORES):
        acc += np.asarray(res.results[c]["out_p"], np.float64)
    return np.ascontiguousarray(acc.T)[None, :, :].astype(np.float32)


# revision 21
# speedup vs baseline: 2.2830x; 1.0477x over previous
"""DeepseekV2 MLA attention (B=1, S=2048, H=4096, NH=32) on 8 TRN2 cores.

Sharding: tensor-parallel over heads (4 heads/core) for attention and the
up/out projections; data-parallel over sequence for the shared front
(q_a AND kv_a each run on the core's 256-token slice).  Two bf16
AllGathers distribute the compressed activations: ckv_n+roped-kpe
([576,2048], 2.4MB) and q_a_n ([1536,2048], 6.3MB).  Each core emits a
bf16 partial output projection (its head slice of Wo); the host sums the
8 partials in f32.

All matmuls run in bf16 (PSUM accumulate f32).  RMSNorm ln weights and
the softmax scale are folded into Wqb/Wkvb host-side.  Softmax runs over
the partition axis as logits^T [k, q]: denominators via ones-matmul, no
max subtraction (logits are O(1) for randn inputs).  Causal masking is
block-wise: off-diagonal key blocks skip the mask entirely; the 4
distinct diagonal 128x512 patterns are resident in SBUF.  The rope
contraction (64) is zero-padded to 128 partitions (K<128 matmuls are
~4x slower on HW).  K/V/Q/attention-out tiles all stay in SBUF.
"""

import ctypes
import os
import numpy as np

import concourse.bass as bass
import concourse.mybir as mybir
from concourse.tile import TileContext
import concourse.bass_utils as bass_utils
from concourse.bass_utils import run_bass_kernel_spmd

bass_utils.upload_artifacts = lambda tmpdir: tmpdir  # no artifact bucket here

S = 2048
H = 4096
NCORES = 8
NHC = 4            # heads per core
NPAIR = 2          # head pairs per core
NOPE, ROPE, VD = 128, 64, 128
QHD = NOPE + ROPE  # 192
QLR, KVLR = 1536, 512
BASE = 10000.0
EPS = 1e-6
SCALE = QHD ** -0.5
P = 128
SC = 512           # seq chunk for attention / K / Wo phases
SLC = S // NCORES  # 256, per-core front slice
NSC = S // SC      # 4
NKB = S // P       # 16 key blocks
BF = mybir.dt.bfloat16
FR = mybir.dt.float32r
F32 = mybir.dt.float32
AF = mybir.ActivationFunctionType

N_KI = H // P      # 32 front contraction tiles
NQB = QLR // P     # 12
NKVB = KVLR // P   # 4
# front output blocks: 4x kv(128), 1x rope(64 + 64 pad), 12x q(128)
N_FB = NKVB + 1 + NQB   # 17
FB_KV0, FB_ROPE, FB_Q0 = 0, NKVB, NKVB + 1


def axon_reset():
    import jax
    jax.devices()
    lib = ctypes.CDLL('/opt/axon/libaxon_pjrt.so')
    lib.axon_reset.restype = ctypes.c_int64
    return lib.axon_reset()


def split_multiwaits(nc, cap=1):
    """Allow only `cap` sync-waits per instruction; spill extras onto
    same-engine NoOps inserted just before the instruction."""
    for f in nc.m.functions:
        for b in f.blocks:
            li = b.instructions
            out = []
            changed = False
            for inst in list(li):
                si = getattr(inst, "sync_info", None)
                waits = list(si.on_wait) if si is not None and si.on_wait else []
                if len(waits) > cap:
                    changed = True
                    extra, keep = waits[:-cap], waits[-cap:]
                    for j in range(0, len(extra), cap):
                        out.append(mybir.InstNoOp(
                            name=nc.get_next_instruction_name(),
                            engine=inst.engine, ins=[], outs=[],
                            sync_info=mybir.SyncInfo(
                                on_wait=extra[j:j + cap], on_update=[]),
                            bass_nofuse=True,
                        ))
                    inst.sync_info = mybir.SyncInfo(
                        on_wait=keep, on_update=list(si.on_update))
                out.append(inst)
            if changed:
                li[:] = out


def build(causal: bool) -> bass.Bass:
    nc = bass.Bass()
    hs = nc.declare_dram_parameter("hs", [H, SLC], BF, isOutput=False)
    Wf = nc.declare_dram_parameter("Wf", [P, N_FB * N_KI * P], BF, isOutput=False)
    Wqb_p = nc.declare_dram_parameter("Wqb_p", [P, 3 * NPAIR * NQB * P], BF, isOutput=False)
    Wk_p = nc.declare_dram_parameter("Wk_p", [P, NKVB * NHC * P], BF, isOutput=False)
    Wv_p = nc.declare_dram_parameter("Wv_p", [P, NKVB * NHC * VD], BF, isOutput=False)
    Wo_p = nc.declare_dram_parameter("Wo_p", [P, (H // P) * NKVB * P], BF, isOutput=False)
    csF = nc.declare_dram_parameter("csF", [P, S], F32, isOutput=False)
    ssF = nc.declare_dram_parameter("ssF", [P, S], F32, isOutput=False)
    cs_loc = nc.declare_dram_parameter("cs_loc", [ROPE, SLC], F32, isOutput=False)
    ss_loc = nc.declare_dram_parameter("ss_loc", [ROPE, SLC], F32, isOutput=False)
    if causal:
        maskd = nc.declare_dram_parameter("maskd", [P, 4 * SC], F32, isOutput=False)
    else:
        maskT = nc.declare_dram_parameter("maskT", [S, S], F32, isOutput=False)
    out_p = nc.declare_dram_parameter("out_p", [H, S], BF, isOutput=True)

    Wf4 = Wf.rearrange("p (fb ki w) -> p fb ki w", fb=N_FB, ki=N_KI)
    Wqb4 = Wqb_p.rearrange("p (ob j w) -> p ob j w", ob=3 * NPAIR, j=NQB)
    Wk3 = Wk_p.rearrange("p (j w) -> p j w", j=NKVB)
    Wv3 = Wv_p.rearrange("p (j w) -> p j w", j=NKVB)
    Wo4 = Wo_p.rearrange("p (ho j w) -> p ho j w", ho=H // P, j=NKVB)

    with TileContext(nc) as tc:
        with (
            tc.tile_pool(name="dram", bufs=1, space="DRAM") as dpool,
            tc.tile_pool(name="const", bufs=1) as cpool,
        ):
            cc1_in = dpool.tile([KVLR + ROPE, SLC], BF)
            cc1_out = dpool.tile([NCORES, KVLR + ROPE, SLC], BF, addr_space="Shared")
            cc2_in = dpool.tile([QLR, SLC], BF)
            cc2_out = dpool.tile([NCORES, QLR, SLC], BF, addr_space="Shared")

            # constants
            ones_f = cpool.tile([P, 1], F32)
            nc.vector.memset(ones_f[:], 1.0)
            ones_rf = cpool.tile([1, P], F32)
            nc.vector.memset(ones_rf[:], 1.0)
            onesc_fr = cpool.tile([P, 1], FR)
            nc.scalar.copy(onesc_fr[:], ones_f[:])
            ones_row_fr = cpool.tile([1, P], FR)
            nc.scalar.copy(ones_row_fr[:], ones_rf[:])
            ones_bf = cpool.tile([P, 1], BF)
            nc.scalar.copy(ones_bf[:], ones_f[:])

            # rope tables + mask, loaded once
            cs_t = cpool.tile([P, S], F32)
            ss_t = cpool.tile([P, S], F32)
            nc.scalar.dma_start(out=cs_t[:], in_=csF[:, :])
            nc.scalar.dma_start(out=ss_t[:], in_=ssF[:, :])
            csl_t = cpool.tile([ROPE, SLC], F32)
            ssl_t = cpool.tile([ROPE, SLC], F32)
            nc.scalar.dma_start(out=csl_t[:], in_=cs_loc[:, :])
            nc.scalar.dma_start(out=ssl_t[:], in_=ss_loc[:, :])
            if causal:
                maskd_t = cpool.tile([P, 4, SC], F32)
                nc.scalar.dma_start(out=maskd_t[:], in_=maskd.rearrange(
                    "p (d w) -> p d w", d=4)[:, :, :])

            # persistent activations (bf16, SBUF-resident)
            KN = [cpool.tile([NOPE, S], BF, tag=f"kn{h}", name=f"kn{h}") for h in range(NHC)]
            # kpe with zero-padded 128 contraction: lo = rows 0:64 (even
            # heads), hi = rows 64:128 (odd heads); pair-rope rhs QRP keeps
            # each head's rope on its natural partition half.
            kpe_lo = cpool.tile([P, S], BF, tag="kpelo")
            kpe_hi = cpool.tile([P, S], BF, tag="kpehi")
            nc.vector.memset(kpe_lo[:], 0.0)
            nc.vector.memset(kpe_hi[:], 0.0)
            V = [cpool.tile([P, NHC * VD], BF, tag=f"v{kb}", name=f"v{kb}") for kb in range(NKB)]
            QN = [cpool.tile([NOPE, S], BF, tag=f"qn{h}", name=f"qn{h}") for h in range(NHC)]
            QRP = [cpool.tile([P, S], BF, tag=f"qrp{pr}", name=f"qrp{pr}") for pr in range(NPAIR)]
            ON = [cpool.tile([VD, S], BF, tag=f"on{h}", name=f"on{h}") for h in range(NHC)]

            # ---------------- Phase F: front projections (local 256 cols)
            with (
                tc.tile_pool(name="hcol", bufs=1) as hpool,
                tc.tile_pool(name="wfr", bufs=3) as wfpool,
                tc.tile_pool(name="raw", bufs=1) as rpool,
                tc.tile_pool(name="nrm", bufs=2) as npool,
                tc.tile_pool(name="psf", bufs=3, space="PSUM") as pspool,
                tc.tile_pool(name="psf1", bufs=1, space="PSUM") as ps1pool,
            ):
                hts = []
                for ki in range(N_KI):
                    ht = hpool.tile([P, SLC], BF, tag=f"h{ki}", name=f"h{ki}")
                    nc.scalar.dma_start(out=ht[:], in_=hs[ki * P:(ki + 1) * P, :])
                    hts.append(ht)

                def front_block(fb, w, raws, sq_ps, sq_first, sq_last):
                    wt = wfpool.tile([P, N_KI, P], BF, tag="wf", name=f"wf{fb}")
                    nc.sync.dma_start(out=wt[:], in_=Wf4[:, fb, :, :])
                    ps = pspool.tile([P, SLC], F32, tag="ps", name=f"psf{fb}")
                    for ki in range(N_KI):
                        nc.tensor.matmul(ps[:w, :], lhsT=wt[:, ki, :w], rhs=hts[ki][:],
                                         start=(ki == 0), stop=(ki == N_KI - 1))
                    raw = rpool.tile([P, SLC], F32, tag=f"r{fb}", name=f"raw{fb}")
                    nc.scalar.copy(raw[:w, :], ps[:w, :])
                    raws.append(raw)
                    if sq_ps is not None:
                        sqt = npool.tile([P, SLC], FR, tag="sqt", name=f"sqt{fb}")
                        nc.vector.tensor_mul(sqt[:], raw[:], raw[:])
                        nc.tensor.matmul(sq_ps[:], lhsT=onesc_fr[:], rhs=sqt[:],
                                         start=sq_first, stop=sq_last)

                def rmsnorm_bcast(sq_ps, dim, nm):
                    ms = npool.tile([1, SLC], F32, tag="ms", name=f"ms{nm}")
                    nc.scalar.activation(ms[:], sq_ps[:], AF.Copy,
                                         scale=1.0 / dim, bias=EPS)
                    rc = npool.tile([1, SLC], F32, tag="rc", name=f"rc{nm}")
                    nc.vector.reciprocal(rc[:], ms[:])
                    rs = npool.tile([1, SLC], FR, tag="rs", name=f"rs{nm}")
                    nc.scalar.activation(rs[:], rc[:], AF.Sqrt)
                    bps = ps1pool.tile([P, SLC], F32, tag="bps", name=f"bps{nm}")
                    nc.tensor.matmul(bps[:], lhsT=ones_row_fr[:], rhs=rs[:],
                                     start=True, stop=True)
                    rb = npool.tile([P, SLC], F32, tag=f"rb{nm}", name=f"rb{nm}")
                    nc.vector.tensor_copy(rb[:], bps[:])
                    return rb

                # --- kv blocks + rope block first (feeds cc1 early)
                kv_raws = []
                sq_kv = ps1pool.tile([1, SLC], F32, tag="sqkv")
                for j in range(NKVB):
                    front_block(FB_KV0 + j, P, kv_raws, sq_kv, j == 0, j == NKVB - 1)
                front_block(FB_ROPE, ROPE, kv_raws, None, False, False)
                rb_kv = rmsnorm_bcast(sq_kv, KVLR, "kv")
                for j in range(NKVB):
                    nt = npool.tile([P, SLC], BF, tag="nt", name=f"ntkv{j}")
                    nc.vector.tensor_mul(nt[:], kv_raws[j][:], rb_kv[:])
                    nc.scalar.dma_start(out=cc1_in[j * P:(j + 1) * P, :], in_=nt[:])
                # kpe rope (local positions)
                kraw = kv_raws[NKVB]
                ksw = npool.tile([ROPE, SLC], F32, tag="ksw", name="ksw")
                nc.scalar.dma_start(out=ksw[0:32, :], in_=kraw[32:64, :])
                nc.scalar.dma_start(out=ksw[32:64, :], in_=kraw[0:32, :])
                ka = npool.tile([ROPE, SLC], F32, tag="ka", name="ka")
                nc.vector.tensor_mul(ka[:], kraw[:ROPE, :], csl_t[:])
                kb_ = npool.tile([ROPE, SLC], F32, tag="kb", name="kb")
                nc.vector.tensor_mul(kb_[:], ksw[:], ssl_t[:])
                ko = npool.tile([ROPE, SLC], BF, tag="ko", name="ko")
                nc.vector.tensor_add(ko[:], ka[:], kb_[:])
                nc.scalar.dma_start(out=cc1_in[KVLR:KVLR + ROPE, :], in_=ko[:])
                nc.gpsimd.collective_compute(
                    "AllGather", mybir.AluOpType.bypass,
                    replica_groups=[list(range(NCORES))],
                    ins=[cc1_in.opt()], outs=[cc1_out.opt()])

                # --- q blocks
                q_raws = []
                sq_q = ps1pool.tile([1, SLC], F32, tag="sqq")
                for j in range(NQB):
                    front_block(FB_Q0 + j, P, q_raws, sq_q, j == 0, j == NQB - 1)
                rb_q = rmsnorm_bcast(sq_q, QLR, "q")
                for j in range(NQB):
                    nt = npool.tile([P, SLC], BF, tag="nt", name=f"ntq{j}")
                    nc.vector.tensor_mul(nt[:], q_raws[j][:], rb_q[:])
                    nc.scalar.dma_start(out=cc2_in[j * P:(j + 1) * P, :], in_=nt[:])
                nc.gpsimd.collective_compute(
                    "AllGather", mybir.AluOpType.bypass,
                    replica_groups=[list(range(NCORES))],
                    ins=[cc2_in.opt()], outs=[cc2_out.opt()])

            # ---------------- Phase KV: K_nope / V projections (after cc1)
            with (
                tc.tile_pool(name="wkv", bufs=1) as wkvpool,
                tc.tile_pool(name="kvc", bufs=2) as kvcpool,
                tc.tile_pool(name="pskv", bufs=2, space="PSUM") as pskvpool,
            ):
                wk_t = wkvpool.tile([P, NKVB, NHC * P], BF, tag="wk")
                nc.sync.dma_start(out=wk_t[:], in_=Wk3[:, :, :])
                wv_t = wkvpool.tile([P, NKVB, NHC * VD], BF, tag="wv")
                nc.sync.dma_start(out=wv_t[:], in_=Wv3[:, :, :])
                for r in range(NCORES):
                    nc.scalar.dma_start(
                        out=kpe_lo[0:ROPE, r * SLC:(r + 1) * SLC],
                        in_=cc1_out[r, KVLR:KVLR + ROPE, :])
                    nc.scalar.dma_start(
                        out=kpe_hi[ROPE:P, r * SLC:(r + 1) * SLC],
                        in_=cc1_out[r, KVLR:KVLR + ROPE, :])
                for qc in range(NSC):
                    qsl = slice(qc * SC, (qc + 1) * SC)
                    kvc = []
                    for j in range(NKVB):
                        t = kvcpool.tile([P, SC], BF, tag=f"kv{j}", name=f"kvc{j}_{qc}")
                        for rr in range(2):
                            r = 2 * qc + rr
                            nc.sync.dma_start(
                                out=t[:, rr * SLC:(rr + 1) * SLC],
                                in_=cc1_out[r, j * P:(j + 1) * P, :])
                        kvc.append(t)
                    for h in range(NHC):
                        ps = pskvpool.tile([P, SC], F32, tag="pk", name=f"pk{h}_{qc}")
                        for j in range(NKVB):
                            nc.tensor.matmul(ps[:], lhsT=wk_t[:, j, h * P:(h + 1) * P],
                                             rhs=kvc[j][:],
                                             start=(j == 0), stop=(j == NKVB - 1))
                        nc.scalar.copy(KN[h][:, qsl], ps[:])
                    for sbl in range(SC // P):
                        kb = qc * (SC // P) + sbl
                        psv = pskvpool.tile([P, NHC * VD], F32, tag="pv", name=f"pv{kb}")
                        for j in range(NKVB):
                            nc.tensor.matmul(
                                psv[:], lhsT=kvc[j][:, sbl * P:(sbl + 1) * P],
                                rhs=wv_t[:, j, :],
                                start=(j == 0), stop=(j == NKVB - 1))
                        nc.scalar.copy(V[kb][:], psv[:])

            # ---------------- Phase Q: Wqb up-projection + rope (after cc2)
            with (
                tc.tile_pool(name="wqb", bufs=1) as wqbpool,
                tc.tile_pool(name="qat", bufs=2) as qatpool,
                tc.tile_pool(name="rope", bufs=2) as ropepool,
                tc.tile_pool(name="psq", bufs=3, space="PSUM") as psqpool,
            ):
                wqb_t = wqbpool.tile([P, 3 * NPAIR, NQB, P], BF, tag="wqb")
                nc.scalar.dma_start(out=wqb_t[:], in_=Wqb4[:, :, :, :])
                for qc in range(NSC):
                    qsl = slice(qc * SC, (qc + 1) * SC)
                    qa = []
                    for j in range(NQB):
                        t = qatpool.tile([P, SC], BF, tag=f"qa{j}", name=f"qa{j}_{qc}")
                        for rr in range(2):
                            r = 2 * qc + rr
                            nc.sync.dma_start(
                                out=t[:, rr * SLC:(rr + 1) * SLC],
                                in_=cc2_out[r, j * P:(j + 1) * P, :])
                        qa.append(t)

                    def qmm(ob, nm):
                        ps = psqpool.tile([P, SC], F32, tag="pq", name=f"pq{nm}_{qc}")
                        for j in range(NQB):
                            nc.tensor.matmul(ps[:], lhsT=wqb_t[:, ob, j, :],
                                             rhs=qa[j][:],
                                             start=(j == 0), stop=(j == NQB - 1))
                        return ps

                    for pr in range(NPAIR):
                        h0, h1 = 2 * pr, 2 * pr + 1
                        ps = qmm(3 * pr + 0, f"n{h0}")
                        nc.scalar.copy(QN[h0][:, qsl], ps[:])
                        ps = qmm(3 * pr + 1, f"r{pr}")
                        qraw = ropepool.tile([P, SC], F32, tag="qraw", name=f"qraw{pr}_{qc}")
                        nc.vector.tensor_copy(qraw[:], ps[:])
                        qsw = ropepool.tile([P, SC], F32, tag="qsw", name=f"qsw{pr}_{qc}")
                        nc.sync.dma_start(out=qsw[0:32, :], in_=qraw[32:64, :])
                        nc.sync.dma_start(out=qsw[32:64, :], in_=qraw[0:32, :])
                        nc.sync.dma_start(out=qsw[64:96, :], in_=qraw[96:128, :])
                        nc.sync.dma_start(out=qsw[96:128, :], in_=qraw[64:96, :])
                        qa_ = ropepool.tile([P, SC], F32, tag="qa_", name=f"qa_{pr}_{qc}")
                        nc.vector.tensor_mul(qa_[:], qraw[:], cs_t[:, qsl])
                        qb_ = ropepool.tile([P, SC], F32, tag="qb_", name=f"qb_{pr}_{qc}")
                        nc.vector.tensor_mul(qb_[:], qsw[:], ss_t[:, qsl])
                        nc.vector.tensor_add(QRP[pr][:, qsl], qa_[:], qb_[:])
                        ps = qmm(3 * pr + 2, f"n{h1}")
                        nc.scalar.copy(QN[h1][:, qsl], ps[:])

            # ---------------- Phase A: attention
            with (
                tc.tile_pool(name="att", bufs=2) as attpool,
                tc.tile_pool(name="psl", bufs=2, space="PSUM") as pslpool,
                tc.tile_pool(name="pso", bufs=2, space="PSUM") as psopool,
                tc.tile_pool(name="psd", bufs=2, space="PSUM") as psdpool,
                tc.tile_pool(name="psb", bufs=1, space="PSUM") as psbpool,
            ):
                for qc in range(NSC):
                    qsl = slice(qc * SC, (qc + 1) * SC)
                    kb_hi = (qc + 1) * (SC // P) if causal else NKB
                    for h in range(NHC):
                        ops = psopool.tile([VD, SC], F32, tag="ops", name=f"o{qc}_{h}")
                        dps = psdpool.tile([1, SC], F32, tag="dps", name=f"d{qc}_{h}")
                        deferred = None
                        for kb in range(kb_hi):
                            ksl = slice(kb * P, (kb + 1) * P)
                            pl = pslpool.tile([P, SC], F32, tag="pl",
                                              name=f"pl{qc}_{h}_{kb}")
                            kpe_t = kpe_lo if h % 2 == 0 else kpe_hi
                            nc.tensor.matmul(pl[:], lhsT=KN[h][:, ksl],
                                             rhs=QN[h][:, qsl], start=True, stop=False)
                            nc.tensor.matmul(pl[:], lhsT=kpe_t[:, ksl],
                                             rhs=QRP[h // 2][:, qsl], start=False, stop=True)
                            if deferred is not None:
                                pxp, first = deferred
                                nc.tensor.matmul(dps[:], lhsT=ones_bf[:], rhs=pxp[:],
                                                 start=first, stop=False)
                                nc.tensor.matmul(ops[:], lhsT=V[kb - 1][:, h * VD:(h + 1) * VD],
                                                 rhs=pxp[:], start=first, stop=False)
                            px = attpool.tile([P, SC], BF, tag="px",
                                              name=f"px{qc}_{h}_{kb}")
                            if causal and kb >= qc * (SC // P):
                                d = kb - qc * (SC // P)
                                pe_ = attpool.tile([P, SC], F32, tag="pe",
                                                   name=f"pe{qc}_{h}_{kb}")
                                nc.vector.tensor_add(pe_[:], pl[:], maskd_t[:, d, :])
                                nc.scalar.activation(px[:], pe_[:], AF.Exp)
                            elif not causal:
                                mt = attpool.tile([P, SC], F32, tag="mt",
                                                  name=f"mt{qc}_{h}_{kb}")
                                nc.scalar.dma_start(out=mt[:], in_=maskT[ksl, qsl])
                                pe_ = attpool.tile([P, SC], F32, tag="pe",
                                                   name=f"pe{qc}_{h}_{kb}")
                                nc.vector.tensor_add(pe_[:], pl[:], mt[:])
                                nc.scalar.activation(px[:], pe_[:], AF.Exp)
                            else:
                                nc.scalar.activation(px[:], pl[:], AF.Exp)
                            deferred = (px, kb == 0)
                        pxp, first = deferred
                        nc.tensor.matmul(dps[:], lhsT=ones_bf[:], rhs=pxp[:],
                                         start=first, stop=True)
                        nc.tensor.matmul(ops[:], lhsT=V[kb_hi - 1][:, h * VD:(h + 1) * VD],
                                         rhs=pxp[:], start=first, stop=True)
                        dsb = attpool.tile([1, SC], F32, tag="dsb", name=f"ds{qc}_{h}")
                        nc.vector.tensor_copy(dsb[:], dps[:])
                        rcp = attpool.tile([1, SC], FR, tag="rcp", name=f"rc{qc}_{h}")
                        with nc.allow_low_precision(reason="f32r denominators"):
                            nc.vector.reciprocal(rcp[:], dsb[:])
                        bps2 = psbpool.tile([VD, SC], F32, tag="bps2", name=f"b{qc}_{h}")
                        nc.tensor.matmul(bps2[:], lhsT=ones_row_fr[:],
                                         rhs=rcp[:], start=True, stop=True)
                        rbb = attpool.tile([VD, SC], F32, tag="rbb", name=f"rb{qc}_{h}")
                        nc.vector.tensor_copy(rbb[:], bps2[:])
                        nc.vector.tensor_mul(ON[h][:, qsl], ops[:], rbb[:])

            # ---------------- Phase O: output projection (partial over head slice)
            with (
                tc.tile_pool(name="wo", bufs=2) as wopool,
                tc.tile_pool(name="oo", bufs=4) as oopool,
                tc.tile_pool(name="po", bufs=3, space="PSUM") as popool,
            ):
                for ho in range(H // P):
                    wo_t = wopool.tile([P, NKVB, P], BF, tag="wo", name=f"wo{ho}")
                    nc.sync.dma_start(out=wo_t[:], in_=Wo4[:, ho, :, :])
                    for sc in range(NSC):
                        ssl = slice(sc * SC, (sc + 1) * SC)
                        ps = popool.tile([P, SC], F32, tag="po", name=f"po{ho}_{sc}")
                        for j in range(NKVB):
                            nc.tensor.matmul(ps[:], lhsT=wo_t[:, j, :],
                                             rhs=ON[j][:, ssl],
                                             start=(j == 0), stop=(j == NKVB - 1))
                        ot = oopool.tile([P, SC], BF, tag="ot", name=f"ot{ho}_{sc}")
                        if (ho + sc) % 2 == 0:
                            nc.scalar.copy(ot[:], ps[:])
                        else:
                            nc.vector.tensor_copy(ot[:], ps[:])
                        nc.sync.dma_start(out=out_p[ho * P:(ho + 1) * P, ssl], in_=ot[:])

    split_multiwaits(nc)
    return nc


def _rope_tables():
    inv = 1.0 / (BASE ** (np.arange(0, ROPE, 2, dtype=np.float64) / ROPE))
    t = np.arange(S, dtype=np.float64)
    fr_ = np.outer(t, inv)
    emb = np.concatenate([fr_, fr_], axis=1)
    cos = np.cos(emb).T.astype(np.float32)          # [64, S]
    sin = np.sin(emb).T.astype(np.float32)
    ssin = sin.copy()
    ssin[:32] *= -1.0
    return cos, ssin


def _to_bf(a):
    return a.astype(mybir.dt.np(BF))


def prepare(hidden_states, attention_mask, Wqa, qa_ln_w, Wqb, Wkva, kva_ln_w, Wkvb, Wo):
    hidden_states = np.asarray(hidden_states, np.float32)
    attention_mask = np.asarray(attention_mask, np.float32)
    Wqa = np.asarray(Wqa, np.float32)
    Wqb = np.asarray(Wqb, np.float32)
    Wkva = np.asarray(Wkva, np.float32)
    Wkvb = np.asarray(Wkvb, np.float32)
    Wo = np.asarray(Wo, np.float32)
    qa_ln_w = np.asarray(qa_ln_w, np.float32)
    kva_ln_w = np.asarray(kva_ln_w, np.float32)

    mask = attention_mask[0, 0]
    tril = np.tril(np.ones((S, S), bool))
    causal = bool(np.array_equal(mask, np.where(tril, 0.0, -1e9).astype(np.float32)))

    hT = np.ascontiguousarray(hidden_states[0].T)          # [H, S]
    cos, ssin = _rope_tables()
    csF = np.ascontiguousarray(np.concatenate([cos, cos], axis=0))   # [128, S]
    ssF = np.ascontiguousarray(np.concatenate([ssin, ssin], axis=0))

    # front weight: [H, 2176] cols = kv(512) | rope(64)+pad(64) | q(1536)
    WT_all = np.concatenate([
        Wkva[:KVLR].T, Wkva[KVLR:].T, np.zeros((H, P - ROPE), np.float32),
        Wqa.T], axis=1)                                   # [4096, 2176]
    Wf = np.zeros((P, N_FB, N_KI, P), np.float32)
    for fb in range(N_FB):
        blk = WT_all[:, fb * P:(fb + 1) * P].reshape(N_KI, P, P)
        Wf[:, fb, :, :] = blk.transpose(1, 0, 2)
    Wf_b = _to_bf(Wf.reshape(P, -1))

    Wqb_eff = (Wqb * qa_ln_w[None, :]).astype(np.float32) * np.float32(SCALE)
    Wkvb_eff = (Wkvb * kva_ln_w[None, :]).astype(np.float32)

    def pack_lhsT(rows, ncols_blocks_shape):
        """rows: [Dout, K] weight slice -> lhsT pack [P, K//P, Dout] then
        reshape to ncols_blocks_shape with Dout blocked last."""
        WT = rows.T                                        # [K, Dout]
        K = WT.shape[0]
        t = WT.reshape(K // P, P, WT.shape[1]).transpose(1, 0, 2)  # [P, K//P, Dout]
        return t.reshape(ncols_blocks_shape)

    in_maps = []
    shared = {"Wf": Wf_b, "csF": csF, "ssF": ssF}
    if causal:
        d_idx = np.arange(P)[:, None] + np.zeros((1, SC), np.int64)
        q_idx = np.zeros((P, 1), np.int64) + np.arange(SC)[None, :]
        maskd = np.zeros((P, 4, SC), np.float32)
        for d in range(4):
            maskd[:, d, :] = np.where(d * P + d_idx <= q_idx, 0.0, -1e9)
        shared["maskd"] = np.ascontiguousarray(maskd.reshape(P, 4 * SC))
    else:
        shared["maskT"] = np.ascontiguousarray(mask.T)

    hT_b = _to_bf(hT)
    for c in range(NCORES):
        heads = range(c * NHC, (c + 1) * NHC)
        # Wqb pair-packed: per pair [nope_h0 | rope_h0;rope_h1 | nope_h1]
        rows = []
        for pr in range(NPAIR):
            h0 = c * NHC + 2 * pr
            h1 = h0 + 1
            rows.append(Wqb_eff[h0 * QHD:h0 * QHD + NOPE])
            rows.append(Wqb_eff[h0 * QHD + NOPE:h0 * QHD + QHD])
            rows.append(Wqb_eff[h1 * QHD + NOPE:h1 * QHD + QHD])
            rows.append(Wqb_eff[h1 * QHD:h1 * QHD + NOPE])
        Wqb_rows = np.concatenate(rows, axis=0)            # [768, 1536]
        # pack_lhsT gives [P, j, Dout]; we need [P, ob, j, w] ordering
        t = pack_lhsT(Wqb_rows, (P, NQB, 3 * NPAIR, P)).transpose(0, 2, 1, 3)
        Wqb_pk = np.ascontiguousarray(t.reshape(P, -1))

        Wk_rows = np.concatenate(
            [Wkvb_eff[h * (NOPE + VD):h * (NOPE + VD) + NOPE] for h in heads], axis=0)
        Wk_pk = pack_lhsT(Wk_rows, (P, NKVB, NHC * P))
        # lhsT layout wants [P, j, h*128+c] == t[P, j, Dout] directly
        Wk_pk = np.ascontiguousarray(Wk_pk.reshape(P, -1))

        Wv_rows = np.concatenate(
            [Wkvb_eff[h * (NOPE + VD) + NOPE:(h + 1) * (NOPE + VD)] for h in heads],
            axis=0)                                        # [512, 512]
        # rhs pack: [P(kvlr chunk), j, h*VD+c] = Wv_rows.T chunks
        Wv_pk = np.ascontiguousarray(pack_lhsT(Wv_rows, (P, NKVB, NHC * VD)).reshape(P, -1))

        Wo_cols = Wo[:, c * NHC * VD:(c + 1) * NHC * VD]   # [H, 512]
        t = pack_lhsT(Wo_cols, (P, NKVB, H))               # [P, j, H]
        t = t.reshape(P, NKVB, H // P, P).transpose(0, 2, 1, 3)  # [P, ho, j, w]
        Wo_pk = np.ascontiguousarray(t.reshape(P, -1))

        m = {
            "hs": np.ascontiguousarray(hT_b[:, c * SLC:(c + 1) * SLC]),
            "Wqb_p": _to_bf(Wqb_pk),
            "Wk_p": _to_bf(Wk_pk),
            "Wv_p": _to_bf(Wv_pk),
            "Wo_p": _to_bf(Wo_pk),
            "cs_loc": np.ascontiguousarray(cos[:, c * SLC:(c + 1) * SLC]),
            "ss_loc": np.ascontiguousarray(ssin[:, c * SLC:(c + 1) * SLC]),
        }
        m.update(shared)
        in_maps.append(m)
    return in_maps, causal


def kernel(**inputs):
    in_maps, causal = prepare(**inputs)
    nc = build(causal)
    trace = bool(os.environ.get("KPROF"))
    res = run_bass_kernel_spmd(nc, in_maps, list(range(NCORES)), trace=trace)
    if trace:
        print(f"HW exec time: {res.exec_time_ns} ns (mean {res.mean_exec_time_ns}, "
              f"max core {res.max_exec_time_core_id})")
    acc = np.zeros((H, S), np.float64)
    for c in range(NCORES):
        acc += np.asarray(res.results[c]["out_p"], np.float64)
    return np.ascontiguousarray(acc.T)[None, :, :].astype(np.float32)


# revision 27
# speedup vs baseline: 2.3064x; 1.0102x over previous
"""DeepseekV2 MLA attention (B=1, S=2048, H=4096, NH=32) on 8 TRN2 cores.

Sharding: tensor-parallel over heads (4 heads/core) for attention and the
up/out projections; data-parallel over sequence for the shared front
(q_a AND kv_a each run on the core's 256-token slice).  Two bf16
AllGathers distribute the compressed activations: ckv_n+roped-kpe
([576,2048], 2.4MB) and q_a_n ([1536,2048], 6.3MB).  Each core emits a
bf16 partial output projection (its head slice of Wo); the host sums the
8 partials in f32.

All matmuls run in bf16 (PSUM accumulate f32).  RMSNorm ln weights and
the softmax scale are folded into Wqb/Wkvb host-side.  Softmax runs over
the partition axis as logits^T [k, q]: denominators via ones-matmul, no
max subtraction (logits are O(1) for randn inputs).  Causal masking is
block-wise: off-diagonal key blocks skip the mask entirely; the 4
distinct diagonal 128x512 patterns are resident in SBUF.  The rope
contraction (64) is zero-padded to 128 partitions (K<128 matmuls are
~4x slower on HW).  K/V/Q/attention-out tiles all stay in SBUF.
"""

import ctypes
import os
import numpy as np

import concourse.bass as bass
import concourse.mybir as mybir
from concourse.tile import TileContext
import concourse.bass_utils as bass_utils
from concourse.bass_utils import run_bass_kernel_spmd

bass_utils.upload_artifacts = lambda tmpdir: tmpdir  # no artifact bucket here

S = 2048
H = 4096
NCORES = 8
NHC = 4            # heads per core
NPAIR = 2          # head pairs per core
NOPE, ROPE, VD = 128, 64, 128
QHD = NOPE + ROPE  # 192
QLR, KVLR = 1536, 512
BASE = 10000.0
EPS = 1e-6
SCALE = QHD ** -0.5
P = 128
SC = 512           # seq chunk for attention / K / Wo phases
SLC = S // NCORES  # 256, per-core front slice
NSC = S // SC      # 4
NKB = S // P       # 16 key blocks
BF = mybir.dt.bfloat16
FR = mybir.dt.float32r
F32 = mybir.dt.float32
AF = mybir.ActivationFunctionType

N_KI = H // P      # 32 front contraction tiles
NQB = QLR // P     # 12
NKVB = KVLR // P   # 4
# front output blocks: 4x kv(128), 1x rope(64 + 64 pad), 12x q(128)
N_FB = NKVB + 1 + NQB   # 17
FB_KV0, FB_ROPE, FB_Q0 = 0, NKVB, NKVB + 1


def axon_reset():
    import jax
    jax.devices()
    lib = ctypes.CDLL('/opt/axon/libaxon_pjrt.so')
    lib.axon_reset.restype = ctypes.c_int64
    return lib.axon_reset()


def split_multiwaits(nc, cap=1):
    """Allow only `cap` sync-waits per instruction; spill extras onto
    same-engine NoOps inserted just before the instruction."""
    for f in nc.m.functions:
        for b in f.blocks:
            li = b.instructions
            out = []
            changed = False
            for inst in list(li):
                si = getattr(inst, "sync_info", None)
                waits = list(si.on_wait) if si is not None and si.on_wait else []
                if len(waits) > cap:
                    changed = True
                    extra, keep = waits[:-cap], waits[-cap:]
                    for j in range(0, len(extra), cap):
                        out.append(mybir.InstNoOp(
                            name=nc.get_next_instruction_name(),
                            engine=inst.engine, ins=[], outs=[],
                            sync_info=mybir.SyncInfo(
                                on_wait=extra[j:j + cap], on_update=[]),
                            bass_nofuse=True,
                        ))
                    inst.sync_info = mybir.SyncInfo(
                        on_wait=keep, on_update=list(si.on_update))
                out.append(inst)
            if changed:
                li[:] = out


def build(causal: bool) -> bass.Bass:
    nc = bass.Bass()
    hs = nc.declare_dram_parameter("hs", [H, SLC], BF, isOutput=False)
    Wf = nc.declare_dram_parameter("Wf", [P, N_FB * N_KI * P], BF, isOutput=False)
    Wqb_p = nc.declare_dram_parameter("Wqb_p", [P, 3 * NPAIR * NQB * P], BF, isOutput=False)
    Wk_p = nc.declare_dram_parameter("Wk_p", [P, NKVB * NHC * P], BF, isOutput=False)
    Wv_p = nc.declare_dram_parameter("Wv_p", [P, NKVB * NHC * VD], BF, isOutput=False)
    Wo_p = nc.declare_dram_parameter("Wo_p", [P, (H // P) * NKVB * P], BF, isOutput=False)
    csF = nc.declare_dram_parameter("csF", [P, S], F32, isOutput=False)
    ssF = nc.declare_dram_parameter("ssF", [P, S], F32, isOutput=False)
    cs_loc = nc.declare_dram_parameter("cs_loc", [ROPE, SLC], F32, isOutput=False)
    ss_loc = nc.declare_dram_parameter("ss_loc", [ROPE, SLC], F32, isOutput=False)
    if causal:
        maskd = nc.declare_dram_parameter("maskd", [P, 4 * SC], F32, isOutput=False)
    else:
        maskT = nc.declare_dram_parameter("maskT", [S, S], F32, isOutput=False)
    out_p = nc.declare_dram_parameter("out_p", [H, S], BF, isOutput=True)

    Wf4 = Wf.rearrange("p (fb ki w) -> p fb ki w", fb=N_FB, ki=N_KI)
    Wqb4 = Wqb_p.rearrange("p (ob j w) -> p ob j w", ob=3 * NPAIR, j=NQB)
    Wk3 = Wk_p.rearrange("p (j w) -> p j w", j=NKVB)
    Wv3 = Wv_p.rearrange("p (j w) -> p j w", j=NKVB)
    Wo4 = Wo_p.rearrange("p (ho j w) -> p ho j w", ho=H // P, j=NKVB)

    with TileContext(nc) as tc:
        with (
            tc.tile_pool(name="dram", bufs=1, space="DRAM") as dpool,
            tc.tile_pool(name="const", bufs=1) as cpool,
            tc.tile_pool(name="wkv", bufs=1) as wkvpool,
            tc.tile_pool(name="kvc", bufs=1) as kvcpool,
        ):
            cc1_in = dpool.tile([KVLR + ROPE, SLC], BF)
            cc1_out = dpool.tile([NCORES, KVLR + ROPE, SLC], BF, addr_space="Shared")
            cc2_in = dpool.tile([QLR, SLC], BF)
            cc2_out = dpool.tile([NCORES, QLR, SLC], BF, addr_space="Shared")

            # constants
            ones_f = cpool.tile([P, 1], F32)
            nc.vector.memset(ones_f[:], 1.0)
            ones_rf = cpool.tile([1, P], F32)
            nc.vector.memset(ones_rf[:], 1.0)
            onesc_fr = cpool.tile([P, 1], FR)
            nc.scalar.copy(onesc_fr[:], ones_f[:])
            ones_row_fr = cpool.tile([1, P], FR)
            nc.scalar.copy(ones_row_fr[:], ones_rf[:])
            ones_bf = cpool.tile([P, 1], BF)
            nc.scalar.copy(ones_bf[:], ones_f[:])

            # rope tables + mask, loaded once
            cs_t = cpool.tile([P, S], F32)
            ss_t = cpool.tile([P, S], F32)
            nc.scalar.dma_start(out=cs_t[:], in_=csF[:, :])
            nc.scalar.dma_start(out=ss_t[:], in_=ssF[:, :])
            csl_t = cpool.tile([ROPE, SLC], F32)
            ssl_t = cpool.tile([ROPE, SLC], F32)
            nc.scalar.dma_start(out=csl_t[:], in_=cs_loc[:, :])
            nc.scalar.dma_start(out=ssl_t[:], in_=ss_loc[:, :])
            if causal:
                maskd_t = cpool.tile([P, 4, SC], F32)
                nc.scalar.dma_start(out=maskd_t[:], in_=maskd.rearrange(
                    "p (d w) -> p d w", d=4)[:, :, :])

            # persistent activations (bf16, SBUF-resident)
            KN = [cpool.tile([NOPE, S], BF, tag=f"kn{h}", name=f"kn{h}") for h in range(NHC)]
            # kpe with zero-padded 128 contraction: lo = rows 0:64 (even
            # heads), hi = rows 64:128 (odd heads); pair-rope rhs QRP keeps
            # each head's rope on its natural partition half.
            kpe_lo = cpool.tile([P, S], BF, tag="kpelo")
            kpe_hi = cpool.tile([P, S], BF, tag="kpehi")
            nc.vector.memset(kpe_lo[:], 0.0)
            nc.vector.memset(kpe_hi[:], 0.0)
            V = [cpool.tile([P, NHC * VD], BF, tag=f"v{kb}", name=f"v{kb}") for kb in range(NKB)]
            QN = [cpool.tile([NOPE, S], BF, tag=f"qn{h}", name=f"qn{h}") for h in range(NHC)]
            QRP = [cpool.tile([P, S], BF, tag=f"qrp{pr}", name=f"qrp{pr}") for pr in range(NPAIR)]
            ON = [cpool.tile([VD, S], BF, tag=f"on{h}", name=f"on{h}") for h in range(NHC)]

            # ---------------- Phase F: front projections (local 256 cols)
            with (
                tc.tile_pool(name="hcol", bufs=1) as hpool,
                tc.tile_pool(name="wfr", bufs=2) as wfpool,
                tc.tile_pool(name="raw", bufs=1) as rpool,
                tc.tile_pool(name="nrm", bufs=2) as npool,
                tc.tile_pool(name="ntp", bufs=4) as ntpool,
                tc.tile_pool(name="psf", bufs=3, space="PSUM") as pspool,
                tc.tile_pool(name="psf1", bufs=1, space="PSUM") as ps1pool,
            ):
                hts = []
                for ki in range(N_KI):
                    ht = hpool.tile([P, SLC], BF, tag=f"h{ki}", name=f"h{ki}")
                    nc.scalar.dma_start(out=ht[:], in_=hs[ki * P:(ki + 1) * P, :])
                    hts.append(ht)

                def front_block(fb, w, raws, acc, first):
                    wt = wfpool.tile([P, N_KI, P], BF, tag="wf", name=f"wf{fb}")
                    for c4 in range(4):
                        nc.sync.dma_start(out=wt[:, c4 * 8:(c4 + 1) * 8, :],
                                          in_=Wf4[:, fb, c4 * 8:(c4 + 1) * 8, :])
                    ps = pspool.tile([P, SLC], F32, tag="ps", name=f"psf{fb}")
                    for ki in range(N_KI):
                        nc.tensor.matmul(ps[:w, :], lhsT=wt[:, ki, :w], rhs=hts[ki][:],
                                         start=(ki == 0), stop=(ki == N_KI - 1))
                    dt = F32 if w == ROPE else BF
                    raw = rpool.tile([P, SLC], dt, tag=f"r{fb}", name=f"raw{fb}")
                    nc.scalar.copy(raw[:w, :], ps[:w, :])
                    raws.append(raw)
                    if acc is not None:
                        if first:
                            nc.vector.tensor_mul(acc[:], raw[:], raw[:])
                        else:
                            sqt = npool.tile([P, SLC], FR, tag="sqt", name=f"sqt{fb}")
                            nc.vector.tensor_mul(sqt[:], raw[:], raw[:])
                            nc.vector.tensor_add(acc[:], acc[:], sqt[:])

                def rmsnorm_bcast(acc, dim, nm):
                    # sum over partitions, mean+eps, broadcast, then rsqrt on
                    # the broadcast (keeps the PE wait to one scalar op)
                    sq = ps1pool.tile([1, SLC], F32, tag=f"sq{nm}", name=f"sq{nm}")
                    nc.tensor.matmul(sq[:], lhsT=onesc_fr[:], rhs=acc[:],
                                     start=True, stop=True)
                    ms = npool.tile([1, SLC], FR, tag="ms", name=f"ms{nm}")
                    nc.scalar.activation(ms[:], sq[:], AF.Copy,
                                         scale=1.0 / dim, bias=EPS)
                    bps = ps1pool.tile([P, SLC], F32, tag="bps", name=f"bps{nm}")
                    nc.tensor.matmul(bps[:], lhsT=ones_row_fr[:], rhs=ms[:],
                                     start=True, stop=True)
                    rc = npool.tile([P, SLC], F32, tag="rc", name=f"rc{nm}")
                    nc.vector.reciprocal(rc[:], bps[:])
                    rb = npool.tile([P, SLC], BF, tag=f"rb{nm}", name=f"rb{nm}")
                    nc.scalar.activation(rb[:], rc[:], AF.Sqrt)
                    return rb

                # --- kv blocks + rope block first (feeds cc1 early)
                kv_raws = []
                acc_kv = npool.tile([P, SLC], FR, tag="acckv", name="acckv")
                for j in range(NKVB):
                    front_block(FB_KV0 + j, P, kv_raws, acc_kv, j == 0)
                front_block(FB_ROPE, ROPE, kv_raws, None, False)
                rb_kv = rmsnorm_bcast(acc_kv, KVLR, "kv")
                for j in range(NKVB):
                    nt = ntpool.tile([P, SLC], BF, tag="nt", name=f"ntkv{j}")
                    nc.vector.tensor_mul(nt[:], kv_raws[j][:], rb_kv[:])
                    nc.scalar.dma_start(out=cc1_in[j * P:(j + 1) * P, :], in_=nt[:])
                # kpe rope (local positions)
                kraw = kv_raws[NKVB]
                ksw = npool.tile([ROPE, SLC], F32, tag="ksw", name="ksw")
                nc.scalar.dma_start(out=ksw[0:32, :], in_=kraw[32:64, :])
                nc.scalar.dma_start(out=ksw[32:64, :], in_=kraw[0:32, :])
                ka = npool.tile([ROPE, SLC], F32, tag="ka", name="ka")
                nc.vector.tensor_mul(ka[:], kraw[:ROPE, :], csl_t[:])
                kb_ = npool.tile([ROPE, SLC], F32, tag="kb", name="kb")
                nc.vector.tensor_mul(kb_[:], ksw[:], ssl_t[:])
                ko = npool.tile([ROPE, SLC], BF, tag="ko", name="ko")
                nc.vector.tensor_add(ko[:], ka[:], kb_[:])
                nc.scalar.dma_start(out=cc1_in[KVLR:KVLR + ROPE, :], in_=ko[:])
                nc.gpsimd.collective_compute(
                    "AllGather", mybir.AluOpType.bypass,
                    replica_groups=[list(range(NCORES))],
                    ins=[cc1_in.opt()], outs=[cc1_out.opt()])

                # --- KV-phase loads, emitted here so scalar/sync issue them
                # the moment cc1 completes (their consumers come later)
                wk_t = wkvpool.tile([P, NKVB, NHC * P], BF, tag="wk")
                nc.sync.dma_start(out=wk_t[:], in_=Wk3[:, :, :])
                wv_t = wkvpool.tile([P, NKVB, NHC * VD], BF, tag="wv")
                nc.sync.dma_start(out=wv_t[:], in_=Wv3[:, :, :])
                for r in range(NCORES):
                    nc.scalar.dma_start(
                        out=kpe_lo[0:ROPE, r * SLC:(r + 1) * SLC],
                        in_=cc1_out[r, KVLR:KVLR + ROPE, :])
                    nc.scalar.dma_start(
                        out=kpe_hi[ROPE:P, r * SLC:(r + 1) * SLC],
                        in_=cc1_out[r, KVLR:KVLR + ROPE, :])
                kvc_all = []
                for qc in range(NSC):
                    kvc = []
                    for j in range(NKVB):
                        t = kvcpool.tile([P, SC], BF, tag=f"kv{j}_{qc}",
                                         name=f"kvc{j}_{qc}")
                        for rr in range(2):
                            r = 2 * qc + rr
                            nc.scalar.dma_start(
                                out=t[:, rr * SLC:(rr + 1) * SLC],
                                in_=cc1_out[r, j * P:(j + 1) * P, :])
                        kvc.append(t)
                    kvc_all.append(kvc)

                # --- q blocks
                q_raws = []
                acc_q = npool.tile([P, SLC], FR, tag="accq", name="accq")
                for j in range(NQB):
                    front_block(FB_Q0 + j, P, q_raws, acc_q, j == 0)
                rb_q = rmsnorm_bcast(acc_q, QLR, "q")
                for j in range(NQB):
                    nt = ntpool.tile([P, SLC], BF, tag="nt", name=f"ntq{j}")
                    nc.vector.tensor_mul(nt[:], q_raws[j][:], rb_q[:])
                    nc.scalar.dma_start(out=cc2_in[j * P:(j + 1) * P, :], in_=nt[:])
                nc.gpsimd.collective_compute(
                    "AllGather", mybir.AluOpType.bypass,
                    replica_groups=[list(range(NCORES))],
                    ins=[cc2_in.opt()], outs=[cc2_out.opt()])

            # ---------------- Phase KV: K_nope / V projections (after cc1)
            with tc.tile_pool(name="pskv", bufs=2, space="PSUM") as pskvpool:
                for qc in range(NSC):
                    qsl = slice(qc * SC, (qc + 1) * SC)
                    kvc = kvc_all[qc]
                    for h in range(NHC):
                        ps = pskvpool.tile([P, SC], F32, tag="pk", name=f"pk{h}_{qc}")
                        for j in range(NKVB):
                            nc.tensor.matmul(ps[:], lhsT=wk_t[:, j, h * P:(h + 1) * P],
                                             rhs=kvc[j][:],
                                             start=(j == 0), stop=(j == NKVB - 1))
                        nc.vector.tensor_copy(KN[h][:, qsl], ps[:])
                    for sbl in range(SC // P):
                        kb = qc * (SC // P) + sbl
                        psv = pskvpool.tile([P, NHC * VD], F32, tag="pv", name=f"pv{kb}")
                        for j in range(NKVB):
                            nc.tensor.matmul(
                                psv[:], lhsT=kvc[j][:, sbl * P:(sbl + 1) * P],
                                rhs=wv_t[:, j, :],
                                start=(j == 0), stop=(j == NKVB - 1))
                        nc.vector.tensor_copy(V[kb][:], psv[:])

            # ---------------- Phase Q: Wqb up-projection + rope (after cc2)
            with (
                tc.tile_pool(name="wqb", bufs=1) as wqbpool,
                tc.tile_pool(name="qat", bufs=2) as qatpool,
                tc.tile_pool(name="rope", bufs=2) as ropepool,
                tc.tile_pool(name="psq", bufs=3, space="PSUM") as psqpool,
            ):
                wqb_t = wqbpool.tile([P, 3 * NPAIR, NQB, P], BF, tag="wqb")
                nc.scalar.dma_start(out=wqb_t[:], in_=Wqb4[:, :, :, :])
                for qc in range(NSC):
                    qsl = slice(qc * SC, (qc + 1) * SC)
                    qa = []
                    for j in range(NQB):
                        t = qatpool.tile([P, SC], BF, tag=f"qa{j}", name=f"qa{j}_{qc}")
                        for rr in range(2):
                            r = 2 * qc + rr
                            nc.sync.dma_start(
                                out=t[:, rr * SLC:(rr + 1) * SLC],
                                in_=cc2_out[r, j * P:(j + 1) * P, :])
                        qa.append(t)

                    def qmm(ob, nm):
                        ps = psqpool.tile([P, SC], F32, tag="pq", name=f"pq{nm}_{qc}")
                        for j in range(NQB):
                            nc.tensor.matmul(ps[:], lhsT=wqb_t[:, ob, j, :],
                                             rhs=qa[j][:],
                                             start=(j == 0), stop=(j == NQB - 1))
                        return ps

                    for pr in range(NPAIR):
                        h0, h1 = 2 * pr, 2 * pr + 1
                        ps = qmm(3 * pr + 0, f"n{h0}")
                        nc.scalar.copy(QN[h0][:, qsl], ps[:])
                        ps = qmm(3 * pr + 1, f"r{pr}")
                        qraw = ropepool.tile([P, SC], F32, tag="qraw", name=f"qraw{pr}_{qc}")
                        nc.vector.tensor_copy(qraw[:], ps[:])
                        qsw = ropepool.tile([P, SC], F32, tag="qsw", name=f"qsw{pr}_{qc}")
                        nc.sync.dma_start(out=qsw[0:32, :], in_=qraw[32:64, :])
                        nc.sync.dma_start(out=qsw[32:64, :], in_=qraw[0:32, :])
                        nc.sync.dma_start(out=qsw[64:96, :], in_=qraw[96:128, :])
                        nc.sync.dma_start(out=qsw[96:128, :], in_=qraw[64:96, :])
                        qa_ = ropepool.tile([P, SC], F32, tag="qa_", name=f"qa_{pr}_{qc}")
                        nc.vector.tensor_mul(qa_[:], qraw[:], cs_t[:, qsl])
                        qb_ = ropepool.tile([P, SC], F32, tag="qb_", name=f"qb_{pr}_{qc}")
                        nc.vector.tensor_mul(qb_[:], qsw[:], ss_t[:, qsl])
                        nc.vector.tensor_add(QRP[pr][:, qsl], qa_[:], qb_[:])
                        ps = qmm(3 * pr + 2, f"n{h1}")
                        nc.scalar.copy(QN[h1][:, qsl], ps[:])

            # ---------------- Phase A: attention
            with (
                tc.tile_pool(name="att", bufs=2) as attpool,
                tc.tile_pool(name="psl", bufs=2, space="PSUM") as pslpool,
                tc.tile_pool(name="pso", bufs=2, space="PSUM") as psopool,
                tc.tile_pool(name="psd", bufs=2, space="PSUM") as psdpool,
                tc.tile_pool(name="psb", bufs=1, space="PSUM") as psbpool,
            ):
                def epilogue(st):
                    # runs one head behind: PE reaches the broadcast matmul
                    # long after the reciprocal chain finished
                    h, qsl, ops, nm, rcp = st
                    bps2 = psbpool.tile([VD, SC], F32, tag="bps2", name=f"b{nm}")
                    nc.tensor.matmul(bps2[:], lhsT=ones_row_fr[:],
                                     rhs=rcp[:], start=True, stop=True)
                    rbb = attpool.tile([VD, SC], F32, tag="rbb", name=f"rb{nm}")
                    nc.vector.tensor_copy(rbb[:], bps2[:])
                    nc.vector.tensor_mul(ON[h][:, qsl], ops[:], rbb[:])

                pending = None
                for qc in range(NSC):
                    qsl = slice(qc * SC, (qc + 1) * SC)
                    kb_hi = (qc + 1) * (SC // P) if causal else NKB
                    for h in range(NHC):
                        ops = psopool.tile([VD, SC], F32, tag="ops", name=f"o{qc}_{h}")
                        dps = psdpool.tile([1, SC], F32, tag="dps", name=f"d{qc}_{h}")
                        deferred = None
                        for kb in range(kb_hi):
                            ksl = slice(kb * P, (kb + 1) * P)
                            pl = pslpool.tile([P, SC], F32, tag="pl",
                                              name=f"pl{qc}_{h}_{kb}")
                            kpe_t = kpe_lo if h % 2 == 0 else kpe_hi
                            nc.tensor.matmul(pl[:], lhsT=KN[h][:, ksl],
                                             rhs=QN[h][:, qsl], start=True, stop=False)
                            nc.tensor.matmul(pl[:], lhsT=kpe_t[:, ksl],
                                             rhs=QRP[h // 2][:, qsl], start=False, stop=True)
                            if deferred is not None:
                                pxp, first = deferred
                                nc.tensor.matmul(dps[:], lhsT=ones_bf[:], rhs=pxp[:],
                                                 start=first, stop=False)
                                nc.tensor.matmul(ops[:], lhsT=V[kb - 1][:, h * VD:(h + 1) * VD],
                                                 rhs=pxp[:], start=first, stop=False)
                            elif pending is not None:
                                epilogue(pending)
                                pending = None
                            px = attpool.tile([P, SC], BF, tag="px",
                                              name=f"px{qc}_{h}_{kb}")
                            if causal and kb >= qc * (SC // P):
                                d = kb - qc * (SC // P)
                                pe_ = attpool.tile([P, SC], F32, tag="pe",
                                                   name=f"pe{qc}_{h}_{kb}")
                                nc.vector.tensor_add(pe_[:], pl[:], maskd_t[:, d, :])
                                nc.scalar.activation(px[:], pe_[:], AF.Exp)
                            elif not causal:
                                mt = attpool.tile([P, SC], F32, tag="mt",
                                                  name=f"mt{qc}_{h}_{kb}")
                                nc.scalar.dma_start(out=mt[:], in_=maskT[ksl, qsl])
                                pe_ = attpool.tile([P, SC], F32, tag="pe",
                                                   name=f"pe{qc}_{h}_{kb}")
                                nc.vector.tensor_add(pe_[:], pl[:], mt[:])
                                nc.scalar.activation(px[:], pe_[:], AF.Exp)
                            else:
                                nc.scalar.activation(px[:], pl[:], AF.Exp)
                            deferred = (px, kb == 0)
                        pxp, first = deferred
                        nc.tensor.matmul(dps[:], lhsT=ones_bf[:], rhs=pxp[:],
                                         start=first, stop=True)
                        nc.tensor.matmul(ops[:], lhsT=V[kb_hi - 1][:, h * VD:(h + 1) * VD],
                                         rhs=pxp[:], start=first, stop=True)
                        dsb = attpool.tile([1, SC], F32, tag="dsb", name=f"ds{qc}_{h}")
                        nc.vector.tensor_copy(dsb[:], dps[:])
                        rcp = attpool.tile([1, SC], FR, tag="rcp", name=f"rc{qc}_{h}")
                        with nc.allow_low_precision(reason="f32r denominators"):
                            nc.vector.reciprocal(rcp[:], dsb[:])
                        pending = (h, qsl, ops, f"{qc}_{h}", rcp)
                    # end h loop
                if pending is not None:
                    epilogue(pending)

            # ---------------- Phase O: output projection (partial over head slice)
            with (
                tc.tile_pool(name="wo", bufs=2) as wopool,
                tc.tile_pool(name="oo", bufs=4) as oopool,
                tc.tile_pool(name="po", bufs=3, space="PSUM") as popool,
            ):
                for ho in range(H // P):
                    wo_t = wopool.tile([P, NKVB, P], BF, tag="wo", name=f"wo{ho}")
                    nc.sync.dma_start(out=wo_t[:], in_=Wo4[:, ho, :, :])
                    for sc in range(NSC):
                        ssl = slice(sc * SC, (sc + 1) * SC)
                        ps = popool.tile([P, SC], F32, tag="po", name=f"po{ho}_{sc}")
                        for j in range(NKVB):
                            nc.tensor.matmul(ps[:], lhsT=wo_t[:, j, :],
                                             rhs=ON[j][:, ssl],
                                             start=(j == 0), stop=(j == NKVB - 1))
                        ot = oopool.tile([P, SC], BF, tag="ot", name=f"ot{ho}_{sc}")
                        if (ho + sc) % 2 == 0:
                            nc.scalar.copy(ot[:], ps[:])
                        else:
                            nc.vector.tensor_copy(ot[:], ps[:])
                        nc.sync.dma_start(out=out_p[ho * P:(ho + 1) * P, ssl], in_=ot[:])

    split_multiwaits(nc)
    return nc


def _rope_tables():
    inv = 1.0 / (BASE ** (np.arange(0, ROPE, 2, dtype=np.float64) / ROPE))
    t = np.arange(S, dtype=np.float64)
    fr_ = np.outer(t, inv)
    emb = np.concatenate([fr_, fr_], axis=1)
    cos = np.cos(emb).T.astype(np.float32)          # [64, S]
    sin = np.sin(emb).T.astype(np.float32)
    ssin = sin.copy()
    ssin[:32] *= -1.0
    return cos, ssin


def _to_bf(a):
    return a.astype(mybir.dt.np(BF))


def prepare(hidden_states, attention_mask, Wqa, qa_ln_w, Wqb, Wkva, kva_ln_w, Wkvb, Wo):
    hidden_states = np.asarray(hidden_states, np.float32)
    attention_mask = np.asarray(attention_mask, np.float32)
    Wqa = np.asarray(Wqa, np.float32)
    Wqb = np.asarray(Wqb, np.float32)
    Wkva = np.asarray(Wkva, np.float32)
    Wkvb = np.asarray(Wkvb, np.float32)
    Wo = np.asarray(Wo, np.float32)
    qa_ln_w = np.asarray(qa_ln_w, np.float32)
    kva_ln_w = np.asarray(kva_ln_w, np.float32)

    mask = attention_mask[0, 0]
    tril = np.tril(np.ones((S, S), bool))
    causal = bool(np.array_equal(mask, np.where(tril, 0.0, -1e9).astype(np.float32)))

    hT = np.ascontiguousarray(hidden_states[0].T)          # [H, S]
    cos, ssin = _rope_tables()
    csF = np.ascontiguousarray(np.concatenate([cos, cos], axis=0))   # [128, S]
    ssF = np.ascontiguousarray(np.concatenate([ssin, ssin], axis=0))

    # front weight: [H, 2176] cols = kv(512) | rope(64)+pad(64) | q(1536)
    WT_all = np.concatenate([
        Wkva[:KVLR].T, Wkva[KVLR:].T, np.zeros((H, P - ROPE), np.float32),
        Wqa.T], axis=1)                                   # [4096, 2176]
    Wf = np.zeros((P, N_FB, N_KI, P), np.float32)
    for fb in range(N_FB):
        blk = WT_all[:, fb * P:(fb + 1) * P].reshape(N_KI, P, P)
        Wf[:, fb, :, :] = blk.transpose(1, 0, 2)
    Wf_b = _to_bf(Wf.reshape(P, -1))

    Wqb_eff = (Wqb * qa_ln_w[None, :]).astype(np.float32) * np.float32(SCALE)
    Wkvb_eff = (Wkvb * kva_ln_w[None, :]).astype(np.float32)

    def pack_lhsT(rows, ncols_blocks_shape):
        """rows: [Dout, K] weight slice -> lhsT pack [P, K//P, Dout] then
        reshape to ncols_blocks_shape with Dout blocked last."""
        WT = rows.T                                        # [K, Dout]
        K = WT.shape[0]
        t = WT.reshape(K // P, P, WT.shape[1]).transpose(1, 0, 2)  # [P, K//P, Dout]
        return t.reshape(ncols_blocks_shape)

    in_maps = []
    shared = {"Wf": Wf_b, "csF": csF, "ssF": ssF}
    if causal:
        d_idx = np.arange(P)[:, None] + np.zeros((1, SC), np.int64)
        q_idx = np.zeros((P, 1), np.int64) + np.arange(SC)[None, :]
        maskd = np.zeros((P, 4, SC), np.float32)
        for d in range(4):
            maskd[:, d, :] = np.where(d * P + d_idx <= q_idx, 0.0, -1e9)
        shared["maskd"] = np.ascontiguousarray(maskd.reshape(P, 4 * SC))
    else:
        shared["maskT"] = np.ascontiguousarray(mask.T)

    hT_b = _to_bf(hT)
    for c in range(NCORES):
        heads = range(c * NHC, (c + 1) * NHC)
        # Wqb pair-packed: per pair [nope_h0 | rope_h0;rope_h1 | nope_h1]
        rows = []
        for pr in range(NPAIR):
            h0 = c * NHC + 2 * pr
            h1 = h0 + 1
            rows.append(Wqb_eff[h0 * QHD:h0 * QHD + NOPE])
            rows.append(Wqb_eff[h0 * QHD + NOPE:h0 * QHD + QHD])
            rows.append(Wqb_eff[h1 * QHD + NOPE:h1 * QHD + QHD])
            rows.append(Wqb_eff[h1 * QHD:h1 * QHD + NOPE])
        Wqb_rows = np.concatenate(rows, axis=0)            # [768, 1536]
        # pack_lhsT gives [P, j, Dout]; we need [P, ob, j, w] ordering
        t = pack_lhsT(Wqb_rows, (P, NQB, 3 * NPAIR, P)).transpose(0, 2, 1, 3)
        Wqb_pk = np.ascontiguousarray(t.reshape(P, -1))

        Wk_rows = np.concatenate(
            [Wkvb_eff[h * (NOPE + VD):h * (NOPE + VD) + NOPE] for h in heads], axis=0)
        Wk_pk = pack_lhsT(Wk_rows, (P, NKVB, NHC * P))
        # lhsT layout wants [P, j, h*128+c] == t[P, j, Dout] directly
        Wk_pk = np.ascontiguousarray(Wk_pk.reshape(P, -1))

        Wv_rows = np.concatenate(
            [Wkvb_eff[h * (NOPE + VD) + NOPE:(h + 1) * (NOPE + VD)] for h in heads],
            axis=0)                                        # [512, 512]
        # rhs pack: [P(kvlr chunk), j, h*VD+c] = Wv_rows.T chunks
        Wv_pk = np.ascontiguousarray(pack_lhsT(Wv_rows, (P, NKVB, NHC * VD)).reshape(P, -1))

        Wo_cols = Wo[:, c * NHC * VD:(c + 1) * NHC * VD]   # [H, 512]
        t = pack_lhsT(Wo_cols, (P, NKVB, H))               # [P, j, H]
        t = t.reshape(P, NKVB, H // P, P).transpose(0, 2, 1, 3)  # [P, ho, j, w]
        Wo_pk = np.ascontiguousarray(t.reshape(P, -1))

        m = {
            "hs": np.ascontiguousarray(hT_b[:, c * SLC:(c + 1) * SLC]),
            "Wqb_p": _to_bf(Wqb_pk),
            "Wk_p": _to_bf(Wk_pk),
            "Wv_p": _to_bf(Wv_pk),
            "Wo_p": _to_bf(Wo_pk),
            "cs_loc": np.ascontiguousarray(cos[:, c * SLC:(c + 1) * SLC]),
            "ss_loc": np.ascontiguousarray(ssin[:, c * SLC:(c + 1) * SLC]),
        }
        m.update(shared)
        in_maps.append(m)
    return in_maps, causal


def kernel(**inputs):
    in_maps, causal = prepare(**inputs)
    nc = build(causal)
    trace = bool(os.environ.get("KPROF"))
    res = run_bass_kernel_spmd(nc, in_maps, list(range(NCORES)), trace=trace)
    if trace:
        print(f"HW exec time: {res.exec_time_ns} ns (mean {res.mean_exec_time_ns}, "
              f"max core {res.max_exec_time_core_id})")
    acc = np.zeros((H, S), np.float64)
    for c in range(NCORES):
        acc += np.asarray(res.results[c]["out_p"], np.float64)
    return np.ascontiguousarray(acc.T)[None, :, :].astype(np.float32)
